# revision 21
# baseline (speedup 1.0000x reference)
"""BertSelfAttention Trainium2 Bass kernel (v4: fully-overlapped pipeline).

Problem: S=2048, B=4, H=1024, NH=16, DH=64, fp32.
  q/k/v = hidden @ W{q,k,v}.T + b   -> softmax((q k^T)/8 + mask) @ v

Sharding over 8 cores: batch (4) x head-group (2 groups of 8 heads).
Each core gets x=[2048,1024] (its batch), W shards [512,1024] (its 8
heads), mask [2048], and produces outT=[512,2048] (feature-major) which
the host transposes and scatters into the full [S,B,H] output.

The kernel is exp-bound: 256 ScalarE activations of [128,1024] at
~1.34us each are the hard floor. v4 hides everything else behind that
stream:
  - x/W are cast fp32->bf16 by gpsimd DMAs; x and group-0 W transposes
    run as PE 4-block packs (bf16, 1 cyc/row); Wq/Wk groups 1-3 go
    through the XBAR dma_start_transpose (measured ~26GB/s serial on
    HW, fine for late-needed weights, frees the PE)
  - a flat 256-iteration stream emits ACT(t), scores(t+1), PV(t-4):
    the 4-tile PV lag (deep ex buffering) lets the exp stream run ahead
    while V projections are still being produced in block 0
  - two independent generators interleave production into the loop:
    gen-A (x transpose packs + K/Q chains, gating scores) and gen-B
    (Wv packs + V chains, gating only PV), each with its own PSUM bank,
    pulled by need() milestones so kt pulls never drag V work in early
  - PV accumulates [1+64, 512] per head with a leading ones-row (the
    softmax denominator lands in PSUM partition 0 for free); the
    epilogue inverts that row in place (RECIPROCAL_APPROX_FAST on
    [1,512], all partition offsets 0 -- offset-mismatched DVE operands
    return garbage on HW), broadcasts it across partitions with
    gpsimd.partition_broadcast, multiplies on DVE, and DMAs the [d, q]
    tile out feature-major on the sync queue; the host transposes
    during gather (off-device). The recip/bcast/mul/DMA part is
    deferred a few iterations so the PE never stalls at boundaries.
"""

import numpy as np

import concourse.bass as bass
import concourse.mybir as mybir
import concourse.tile as tile
from concourse import bacc
from concourse.bass_utils import run_bass_kernel_spmd
from concourse.masks import make_identity

F32 = mybir.dt.float32
BF16 = mybir.dt.bfloat16
AF = mybir.ActivationFunctionType

S, B, H, NH, DH = 2048, 4, 1024, 16, 64
N_CORES = 8
HPC = 8            # heads per core
DPC = HPC * DH     # 512 output features per core
SC = S // 128      # 16 s-chunks
FC = H // 128      # 8 feature chunks
QG = S // 512      # 4 query groups
KC = S // 128      # 16 key chunks
NG = 4             # head-pair groups per core
LAG = 4            # PV trails ACT by this many tiles
AHEAD = 4          # scores are emitted this many tiles ahead of ACT


def _emit(ctx, tc, nc, x, mask, wq, bq, wk, bk, wv, bv, outT):
    import os
    dbg_aps = getattr(nc, "_dbg_aps", None) if os.environ.get("K_DEBUG") else None

    const_p = ctx.enter_context(tc.tile_pool(name="const", bufs=1))
    xstage_p = ctx.enter_context(tc.tile_pool(name="xstage", bufs=10))
    wstage_p = ctx.enter_context(tc.tile_pool(name="wstage", bufs=12))
    xt_p = ctx.enter_context(tc.tile_pool(name="xt", bufs=1))
    wvt_p = ctx.enter_context(tc.tile_pool(name="wvt", bufs=1))
    wt_p = ctx.enter_context(tc.tile_pool(name="wt", bufs=8))
    v_p = ctx.enter_context(tc.tile_pool(name="v", bufs=SC))
    qkt_p = ctx.enter_context(tc.tile_pool(name="qkt", bufs=4))
    ex_p = ctx.enter_context(tc.tile_pool(name="ex", bufs=10))
    ctxs_p = ctx.enter_context(tc.tile_pool(name="ctxs", bufs=2))
    rec_p = ctx.enter_context(tc.tile_pool(name="rec", bufs=2))
    bcs_p = ctx.enter_context(tc.tile_pool(name="bcs", bufs=2))
    outt_p = ctx.enter_context(tc.tile_pool(name="outt", bufs=4))

    # psum (8 banks): mm 2x2 (score tiles) + ctx 2x1 (PV accumulators /
    # prologue packs+chains) + qa 1 (gen-A) + qb 1 (gen-B)
    psum_mm = ctx.enter_context(tc.tile_pool(name="psmm", bufs=2, space="PSUM"))
    psum_ctx = ctx.enter_context(tc.tile_pool(name="psctx", bufs=2, space="PSUM"))
    psum_qa = ctx.enter_context(tc.tile_pool(name="psqa", bufs=1, space="PSUM"))
    psum_qb = ctx.enter_context(tc.tile_pool(name="psqb", bufs=1, space="PSUM"))

    # ---- constants ----
    mask_sb = const_p.tile([128, KC], F32)
    nc.sync.dma_start(out=mask_sb, in_=mask.rearrange("(c p) -> p c", p=128))

    ident = const_p.tile([128, 128], F32)
    make_identity(nc, ident)
    ident_bf = const_p.tile([128, 128], BF16)
    nc.vector.tensor_copy(ident_bf, ident)

    ones_f = const_p.tile([1, 512], F32)
    nc.vector.memset(ones_f, 1.0)
    ones512 = const_p.tile([1, 512], BF16)
    nc.vector.tensor_copy(ones512, ones_f)
    ones_col_f = const_p.tile([128, HPC, 1], F32)
    nc.vector.memset(ones_col_f, 1.0)
    bq_sb = const_p.tile([1, DPC], BF16)
    nc.gpsimd.dma_start(out=bq_sb, in_=bq.rearrange("(a f) -> a f", a=1))
    bk_sb = const_p.tile([1, DPC], BF16)
    nc.gpsimd.dma_start(out=bk_sb, in_=bk.rearrange("(a f) -> a f", a=1))
    bv_sb = const_p.tile([1, DPC], BF16)
    nc.gpsimd.dma_start(out=bv_sb, in_=bv.rearrange("(a f) -> a f", a=1))

    # ---- staging casts (gpsimd DMA, fp32->bf16), priority order ----
    xt = xt_p.tile([128, FC, S], BF16)
    wvt = wvt_p.tile([128, FC, DPC], BF16)
    wqts = [wt_p.tile([128, FC, 128], BF16, tag="wt", name=f"wqt{g}")
            for g in range(NG)]
    wkts = [wt_p.tile([128, FC, 128], BF16, tag="wt", name=f"wkt{g}")
            for g in range(NG)]

    x_nat = [xstage_p.tile([128, H], BF16, tag="xs", name=f"xn{sc}")
             for sc in range(SC)]
    w_nat = {}
    cast_jobs = []

    def stage_w(wsrc, key, g):
        nat = wstage_p.tile([128, H], BF16, tag="ws", name=f"wn_{key}{g}")
        w_nat[(key, g)] = nat
        cast_jobs.append((nat, wsrc[g * 128:(g + 1) * 128, :]))

    stage_w(wq, "q", 0)
    stage_w(wk, "k", 0)
    for sc in range(4):
        cast_jobs.append((x_nat[sc], x[sc * 128:(sc + 1) * 128, :]))
    for dc in range(4):
        stage_w(wv, "v", dc)
    for g in range(1, NG):
        stage_w(wq, "q", g)
        stage_w(wk, "k", g)
    for sc in range(4, SC):
        cast_jobs.append((x_nat[sc], x[sc * 128:(sc + 1) * 128, :]))

    for nat, src in cast_jobs:
        nc.gpsimd.dma_start(out=nat, in_=src)

    # Wq/Wk groups 1-3 transposed by the (slow but off-engine) XBAR DMA.
    for g in (1, 2, 3):
        for key, dst in (("q", wqts[g]), ("k", wkts[g])):
            nat = w_nat[(key, g)]
            for fc in range(FC):
                nc.sync.dma_start_transpose(dst[:, fc, :],
                                            nat[:, fc * 128:(fc + 1) * 128])

    # ---- PE transpose packs (bf16, via psum) ----
    def tp_pack(dst_view, src_nat, fc0, pool, tag):
        pt = pool.tile([128, 4, 128], BF16, tag=tag, name="pt")
        for j in range(4):
            fc = fc0 + j
            nc.tensor.transpose(pt[:, j, :],
                                src_nat[:, fc * 128:(fc + 1) * 128], ident_bf)
            yield
        nc.vector.tensor_copy(dst_view, pt)
        yield

    def tp_x(sc, pool, tag):
        for fc0 in (0, 4):
            yield from tp_pack(xt[:, fc0:fc0 + 4, sc * 128:(sc + 1) * 128],
                               x_nat[sc], fc0, pool, tag)

    def tp_wqk(key, g, pool, tag):
        dst = wqts[g] if key == "q" else wkts[g]
        for fc0 in (0, 4):
            yield from tp_pack(dst[:, fc0:fc0 + 4, :], w_nat[(key, g)],
                               fc0, pool, tag)

    def tp_wv(dc, pool, tag):
        for fc0 in (0, 4):
            yield from tp_pack(wvt[:, fc0:fc0 + 4, dc * 128:(dc + 1) * 128],
                               w_nat[("v", dc)], fc0, pool, tag)

    # ---- projection chains ----
    # v_sb layout: [:, h, 0] = ones (denominator row), [:, h, 1:65] = V
    v_sb = [v_p.tile([128, HPC, DH + 1], BF16, tag="v", name=f"v{sc}")
            for sc in range(SC)]
    qts = {}
    kts = {}

    def get_qkt(kind, g):
        d = qts if kind == "qt" else kts
        if g not in d:
            d[g] = qkt_p.tile([128, S], BF16, tag="qkt", name=f"{kind}{g}")
        return d[g]

    def v_chain(sc, pool, tag):
        vp = pool.tile([128, DPC], F32, tag=tag, name=f"vp{sc}")
        for fc in range(FC):
            nc.tensor.matmul(vp, xt[:, fc, sc * 128:(sc + 1) * 128],
                             wvt[:, fc, :], start=(fc == 0), stop=False)
            yield
        nc.tensor.matmul(vp, ones512[:, 0:128], bv_sb, start=False, stop=True)
        nc.gpsimd.tensor_copy(v_sb[sc][:, :, 0:1], ones_col_f)
        yield
        nc.vector.tensor_copy(v_sb[sc][:, :, 1:DH + 1],
                              vp.rearrange("p (h d) -> p h d", d=DH))
        yield

    def qk_chain(kind, g, sg, pool, tag):
        bias_sb = bq_sb if kind == "qt" else bk_sb
        wt_src = wqts[g] if kind == "qt" else wkts[g]
        qk_dst = get_qkt(kind, g)
        ssl = slice(sg * 512, (sg + 1) * 512)
        qp = pool.tile([128, 512], F32, tag=tag, name=f"{kind}{g}s{sg}p")
        for fc in range(FC):
            nc.tensor.matmul(qp, wt_src[:, fc, :], xt[:, fc, ssl],
                             start=(fc == 0), stop=False)
            yield
        nc.tensor.matmul(qp, bias_sb[:, g * 128:(g + 1) * 128],
                         ones512, start=False, stop=True)
        yield
        nc.vector.tensor_copy(qk_dst[:, ssl], qp)
        yield

    done = set()

    def run_now(gen_):
        for _ in gen_:
            pass

    # ---- prologue: minimum to start the exp stream ----
    run_now(tp_wqk("q", 0, psum_ctx, "ctx"))
    run_now(tp_wqk("k", 0, psum_ctx, "ctx"))
    for sc in range(4):
        run_now(tp_x(sc, psum_ctx, "ctx"))
    run_now(qk_chain("kt", 0, 0, psum_ctx, "ctx"))
    run_now(qk_chain("qt", 0, 0, psum_ctx, "ctx"))
    done.update({"kt0s0", "qt0s0"})

    # ---- generators: A gates scores (kt/qt), B gates PV (v) ----
    plan_a = [("x", 4), ("x", 5), ("x", 6), ("x", 7), ("kt", 0, 1),
              ("x", 8), ("x", 9), ("x", 10), ("x", 11), ("kt", 0, 2),
              ("x", 12), ("x", 13), ("x", 14), ("x", 15), ("kt", 0, 3),
              ("qt", 0, 1), ("qt", 0, 2), ("qt", 0, 3)]
    for g in range(1, NG):
        plan_a += [("kt", g, 0), ("qt", g, 0), ("kt", g, 1), ("kt", g, 2),
                   ("kt", g, 3), ("qt", g, 1), ("qt", g, 2), ("qt", g, 3)]
    plan_b = ([("wv", dc) for dc in range(4)] +
              [("v", sc) for sc in range(SC)])

    def run_plan(plan, pool, tag):
        for item in plan:
            if item[0] == "x":
                yield from tp_x(item[1], pool, tag)
                done.add(f"x{item[1]}")
            elif item[0] == "wv":
                yield from tp_wv(item[1], pool, tag)
            elif item[0] == "v":
                sc = item[1]
                if sc >= 4:
                    # xt[sc] comes from gen-A: force its pack to be
                    # emitted first (cross-generator RAW dependency)
                    need(0, f"x{sc}")
                yield from v_chain(sc, pool, tag)
                done.add(f"v{sc}")
            else:
                kind, g, sg = item
                yield from qk_chain(kind, g, sg, pool, tag)
                done.add(f"{kind}{g}s{sg}")

    gens = [run_plan(plan_a, psum_qa, "qa"), run_plan(plan_b, psum_qb, "qb")]

    def drive(n):
        # round-robin both generators
        for _ in range(n):
            alive = [g for g in gens if g is not None]
            if not alive:
                return
            for idx in range(2):
                if gens[idx] is None:
                    continue
                try:
                    next(gens[idx])
                except StopIteration:
                    gens[idx] = None

    def need(idx, *products):
        while gens[idx] is not None and not all(p in done for p in products):
            try:
                next(gens[idx])
            except StopIteration:
                gens[idx] = None

    # ---- attention: flat pipelined stream ----
    blocks = [(g2, qg) for g2 in range(NG) for qg in range(QG)]
    T = len(blocks) * KC
    pend_st = {}
    cur_cp = {}

    def emit_scores(t):
        bi, kc = divmod(t, KC)
        g2, qg = blocks[bi]
        qt, kt = get_qkt("qt", g2), get_qkt("kt", g2)
        ksl = slice(kc * 128, (kc + 1) * 128)
        qsl = slice(qg * 512, (qg + 1) * 512)
        st = psum_mm.tile([128, 2, 512], F32, tag="mm")
        nc.tensor.matmul(st[:, 0, :], kt[0:64, ksl], qt[0:64, qsl],
                         start=True, stop=True)
        nc.tensor.matmul(st[:, 1, :], kt[64:128, ksl], qt[64:128, qsl],
                         start=True, stop=True)
        pend_st[t] = st

    pend_ex = {}
    post = []   # deferred epilogue closures (recip/bcast/mul/dma)

    if dbg_aps:
        dbg2_p = ctx.enter_context(tc.tile_pool(name="dbgt", bufs=1))
        dbg_ex = dbg2_p.tile([128, 2, 512], BF16, tag="dx", name="dbg_ex")
        dbg_ctxs = dbg2_p.tile([DH + 1, 512], F32, tag="dc", name="dbg_ctxs")
        dbg_rec = dbg2_p.tile([1, 512], F32, tag="dr", name="dbg_rec")
        dbg_bc = dbg2_p.tile([DH + 1, 512], F32, tag="db", name="dbg_bc")

    def epilogue_a(bi, cp0, cp1):
        g2, qg = blocks[bi]
        qsl = slice(qg * 512, (qg + 1) * 512)
        for h_loc, cp in ((0, cp0), (1, cp1)):
            h = 2 * g2 + h_loc
            ctxs = ctxs_p.tile([DH + 1, 512], F32, tag="ctxs")
            nc.vector.tensor_copy(ctxs, cp)
            first = (bi == 0 and h_loc == 0)
            if dbg_aps and first:
                nc.vector.tensor_copy(dbg_ctxs, ctxs)

            def fin(h=h, ctxs=ctxs, qsl=qsl, first=first):
                rec = rec_p.tile([1, 512], F32, tag="rec")
                nc.vector.reciprocal_approx_fast(rec, ctxs[0:1, :])
                bc = bcs_p.tile([DH + 1, 512], F32, tag="bc")
                nc.gpsimd.partition_broadcast(bc, rec)
                ot = outt_p.tile([DH + 1, 512], F32, tag="outt")
                nc.vector.tensor_mul(ot, ctxs, bc)
                nc.sync.dma_start(out=outT[h * DH:(h + 1) * DH, qsl],
                                  in_=ot[1:DH + 1, :])
                if dbg_aps and first:
                    nc.vector.tensor_copy(dbg_rec, rec)
                    nc.vector.tensor_copy(dbg_bc, bc)

            post.append(fin)

    for k in range(AHEAD):
        emit_scores(k)
    drive(12)
    for t in range(T + LAG):
        if t < T:
            bi, kc = divmod(t, KC)
            st = pend_st.pop(t)
            ex = ex_p.tile([128, 2, 512], BF16, tag="ex")
            nc.scalar.activation(ex.rearrange("p a b -> p (a b)"),
                                 st.rearrange("p a b -> p (a b)"),
                                 AF.Exp, bias=mask_sb[:, kc:kc + 1],
                                 scale=1.0 / np.sqrt(DH))
            pend_ex[t] = ex
            if dbg_aps and t == 0:
                nc.gpsimd.tensor_copy(dbg_ex, ex)
            if t + AHEAD < T:
                nbi, nkc = divmod(t + AHEAD, KC)
                ng2, nqg = blocks[nbi]
                need(0, f"kt{ng2}s{nkc // 4}", f"qt{ng2}s{nqg}")
                emit_scores(t + AHEAD)
        pt_ = t - LAG
        if pt_ >= 0:
            pbi, pkc = divmod(pt_, KC)
            pg2, _ = blocks[pbi]
            if pkc == 0:
                cpa = psum_ctx.tile([DH + 1, 512], F32, tag="ctx", name="cpa")
                cpb = psum_ctx.tile([DH + 1, 512], F32, tag="ctx", name="cpb")
                cur_cp[pbi] = (cpa, cpb)
            cp0, cp1 = cur_cp[pbi]
            if pbi == 0:
                need(1, f"v{pkc}")
            ex = pend_ex.pop(pt_)
            nc.tensor.matmul(cp0, v_sb[pkc][:, 2 * pg2, :], ex[:, 0, :],
                             start=(pkc == 0), stop=(pkc == KC - 1))
            nc.tensor.matmul(cp1, v_sb[pkc][:, 2 * pg2 + 1, :], ex[:, 1, :],
                             start=(pkc == 0), stop=(pkc == KC - 1))
            if pkc == KC - 1:
                epilogue_a(pbi, cp0, cp1)
                del cur_cp[pbi]
        if (t % KC) in (9, 13) and post:
            post.pop(0)()
        drive(2)

    while post:
        post.pop(0)()
    for idx in range(2):
        while gens[idx] is not None:
            try:
                next(gens[idx])
            except StopIteration:
                gens[idx] = None

    if dbg_aps:
        nc.sync.dma_start(out=dbg_aps["xt"], in_=xt)
        nc.sync.dma_start(out=dbg_aps["qt0"], in_=qts[0])
        nc.sync.dma_start(out=dbg_aps["kt0"], in_=kts[0])
        nc.sync.dma_start(out=dbg_aps["v0"], in_=v_sb[0])
        nc.sync.dma_start(out=dbg_aps["ex00"], in_=dbg_ex)
        nc.sync.dma_start(out=dbg_aps["ctxs0"], in_=dbg_ctxs)
        nc.sync.dma_start(out=dbg_aps["rec0"], in_=dbg_rec)
        nc.sync.dma_start(out=dbg_aps["bc0"], in_=dbg_bc)


def build_program():
    nc = bacc.Bacc("TRN2", target_bir_lowering=False, debug=False)
    x = nc.dram_tensor("x", [S, H], F32, kind="ExternalInput").ap()
    mask = nc.dram_tensor("mask", [S], F32, kind="ExternalInput").ap()
    wq = nc.dram_tensor("wq", [DPC, H], F32, kind="ExternalInput").ap()
    bq = nc.dram_tensor("bq", [DPC], F32, kind="ExternalInput").ap()
    wk = nc.dram_tensor("wk", [DPC, H], F32, kind="ExternalInput").ap()
    bk = nc.dram_tensor("bk", [DPC], F32, kind="ExternalInput").ap()
    wv = nc.dram_tensor("wv", [DPC, H], F32, kind="ExternalInput").ap()
    bv = nc.dram_tensor("bv", [DPC], F32, kind="ExternalInput").ap()
    outT = nc.dram_tensor("outT", [DPC, S], F32, kind="ExternalOutput").ap()

    import os
    if os.environ.get("K_DEBUG"):
        nc._dbg_aps = {
            "xt": nc.dram_tensor("xt_dbg", [128, FC, S], BF16,
                                 kind="ExternalOutput").ap(),
            "qt0": nc.dram_tensor("qt0_dbg", [128, S], BF16,
                                  kind="ExternalOutput").ap(),
            "kt0": nc.dram_tensor("kt0_dbg", [128, S], BF16,
                                  kind="ExternalOutput").ap(),
            "v0": nc.dram_tensor("v0_dbg", [128, HPC, DH + 1], BF16,
                                 kind="ExternalOutput").ap(),
            "ex00": nc.dram_tensor("ex00_dbg", [128, 2, 512], BF16,
                                   kind="ExternalOutput").ap(),
            "ctxs0": nc.dram_tensor("ctxs0_dbg", [DH + 1, 512], F32,
                                    kind="ExternalOutput").ap(),
            "rec0": nc.dram_tensor("rec0_dbg", [1, 512], F32,
                                   kind="ExternalOutput").ap(),
            "bc0": nc.dram_tensor("bc0_dbg", [DH + 1, 512], F32,
                                  kind="ExternalOutput").ap(),
        }

    from contextlib import ExitStack
    with tile.TileContext(nc) as tc:
        with ExitStack() as ctx:
            _emit(ctx, tc, nc, x, mask, wq, bq, wk, bk, wv, bv, outT)
    nc.compile()
    return nc


_NC_CACHE = None


def make_in_maps(hidden_states, attention_mask, Wq, bq, Wk, bk, Wv, bv):
    hs = np.asarray(hidden_states, dtype=np.float32)
    am = np.asarray(attention_mask, dtype=np.float32)
    ws = {k: np.asarray(v, dtype=np.float32)
          for k, v in (("wq", Wq), ("bq", bq), ("wk", Wk),
                       ("bk", bk), ("wv", Wv), ("bv", bv))}
    in_maps = []
    for c in range(N_CORES):
        b, g = divmod(c, 2)
        sl = slice(g * DPC, (g + 1) * DPC)
        in_maps.append({
            "x": np.ascontiguousarray(hs[:, b, :]),
            "mask": np.ascontiguousarray(am[b, 0, 0, :]),
            "wq": np.ascontiguousarray(ws["wq"][sl]),
            "bq": np.ascontiguousarray(ws["bq"][sl]),
            "wk": np.ascontiguousarray(ws["wk"][sl]),
            "bk": np.ascontiguousarray(ws["bk"][sl]),
            "wv": np.ascontiguousarray(ws["wv"][sl]),
            "bv": np.ascontiguousarray(ws["bv"][sl]),
        })
    return in_maps


def gather_out(results):
    out = np.empty((S, B, H), np.float32)
    for c in range(N_CORES):
        b, g = divmod(c, 2)
        out[:, b, g * DPC:(g + 1) * DPC] = results[c]["outT"].T
    return out


def kernel(hidden_states, attention_mask, Wq, bq, Wk, bk, Wv, bv):
    global _NC_CACHE
    if _NC_CACHE is None:
        _NC_CACHE = build_program()
    in_maps = make_in_maps(hidden_states, attention_mask,
                           Wq, bq, Wk, bk, Wv, bv)
    res = run_bass_kernel_spmd(_NC_CACHE, in_maps, list(range(N_CORES)))
    return gather_out(res.results)


# revision 28
# speedup vs baseline: 1.2179x; 1.2179x over previous
"""BertSelfAttention Trainium2 Bass kernel (v4: fully-overlapped pipeline).

Problem: S=2048, B=4, H=1024, NH=16, DH=64, fp32.
  q/k/v = hidden @ W{q,k,v}.T + b   -> softmax((q k^T)/8 + mask) @ v

Sharding over 8 cores: batch (4) x head-group (2 groups of 8 heads).
Each core gets x=[2048,1024] (its batch), W shards [512,1024] (its 8
heads), mask [2048], and produces outT=[512,2048] (feature-major) which
the host transposes and scatters into the full [S,B,H] output.

The kernel is exp-bound: 256 ScalarE activations of [128,1024] at
~1.34us each are the hard floor. v4 hides everything else behind that
stream:
  - x/W are cast fp32->bf16 by gpsimd DMAs; x and group-0 W transposes
    run as PE 4-block packs (bf16, 1 cyc/row); Wq/Wk groups 1-3 go
    through the XBAR dma_start_transpose (measured ~26GB/s serial on
    HW, fine for late-needed weights, frees the PE)
  - a flat 256-iteration stream emits ACT(t), scores(t+1), PV(t-4):
    the 4-tile PV lag (deep ex buffering) lets the exp stream run ahead
    while V projections are still being produced in block 0
  - two independent generators interleave production into the loop:
    gen-A (x transpose packs + K/Q chains, gating scores) and gen-B
    (Wv packs + V chains, gating only PV), each with its own PSUM bank,
    pulled by need() milestones so kt pulls never drag V work in early
  - PV accumulates [1+64, 512] per head with a leading ones-row (the
    softmax denominator lands in PSUM partition 0 for free); the
    epilogue inverts that row in place (RECIPROCAL_APPROX_FAST on
    [1,512], all partition offsets 0 -- offset-mismatched DVE operands
    return garbage on HW), broadcasts it across partitions with
    gpsimd.partition_broadcast, multiplies on DVE, and DMAs the [d, q]
    tile out feature-major on the sync queue; the host transposes
    during gather (off-device). The recip/bcast/mul/DMA part is
    deferred a few iterations so the PE never stalls at boundaries.
"""

import numpy as np

import concourse.bass as bass
import concourse.mybir as mybir
import concourse.tile as tile
from concourse import bacc
from concourse.bass_utils import run_bass_kernel_spmd
from concourse.masks import make_identity

F32 = mybir.dt.float32
BF16 = mybir.dt.bfloat16
AF = mybir.ActivationFunctionType

S, B, H, NH, DH = 2048, 4, 1024, 16, 64
N_CORES = 8
HPC = 8            # heads per core
DPC = HPC * DH     # 512 output features per core
SC = S // 128      # 16 s-chunks
FC = H // 128      # 8 feature chunks
QG = S // 512      # 4 query groups
KC = S // 128      # 16 key chunks
NG = 4             # head-pair groups per core
LAG = 4            # PV trails ACT by this many tiles
AHEAD = 4          # scores are emitted this many tiles ahead of ACT


def _emit(ctx, tc, nc, x, mask, wq, bq, wk, bk, wv, bv, outT):
    import os
    dbg_aps = getattr(nc, "_dbg_aps", None) if os.environ.get("K_DEBUG") else None

    const_p = ctx.enter_context(tc.tile_pool(name="const", bufs=1))
    xstage_p = ctx.enter_context(tc.tile_pool(name="xstage", bufs=8))
    wstage_p = ctx.enter_context(tc.tile_pool(name="wstage", bufs=6))
    wstgb_p = ctx.enter_context(tc.tile_pool(name="wstgb", bufs=6))
    xt_p = ctx.enter_context(tc.tile_pool(name="xt", bufs=1))
    wvt_p = ctx.enter_context(tc.tile_pool(name="wvt", bufs=1))
    wt_p = ctx.enter_context(tc.tile_pool(name="wt", bufs=8))
    v_p = ctx.enter_context(tc.tile_pool(name="v", bufs=SC))
    qkt_p = ctx.enter_context(tc.tile_pool(name="qkt", bufs=4))
    ex_p = ctx.enter_context(tc.tile_pool(name="ex", bufs=10))
    ctxs_p = ctx.enter_context(tc.tile_pool(name="ctxs", bufs=2))
    rec_p = ctx.enter_context(tc.tile_pool(name="rec", bufs=2))
    bcs_p = ctx.enter_context(tc.tile_pool(name="bcs", bufs=2))
    outt_p = ctx.enter_context(tc.tile_pool(name="outt", bufs=4))

    # psum (8 banks): mm 2x2 (score tiles) + ctx 2x1 (PV accumulators /
    # prologue packs+chains) + qa 1 (gen-A) + qb 1 (gen-B)
    psum_mm = ctx.enter_context(tc.tile_pool(name="psmm", bufs=2, space="PSUM"))
    psum_ctx = ctx.enter_context(tc.tile_pool(name="psctx", bufs=2, space="PSUM"))
    psum_qa = ctx.enter_context(tc.tile_pool(name="psqa", bufs=1, space="PSUM"))
    psum_qb = ctx.enter_context(tc.tile_pool(name="psqb", bufs=1, space="PSUM"))

    # ---- constants ----
    mask_sb = const_p.tile([128, KC], F32)
    nc.sync.dma_start(out=mask_sb, in_=mask.rearrange("(c p) -> p c", p=128))

    ident = const_p.tile([128, 128], F32)
    make_identity(nc, ident)

    ones_f = const_p.tile([1, 512], F32)
    nc.vector.memset(ones_f, 1.0)
    ones512 = const_p.tile([1, 512], BF16)
    nc.vector.tensor_copy(ones512, ones_f)
    ones_col_f = const_p.tile([128, HPC, 1], F32)
    nc.vector.memset(ones_col_f, 1.0)
    bq_sb = const_p.tile([1, DPC], BF16)
    nc.gpsimd.dma_start(out=bq_sb, in_=bq.rearrange("(a f) -> a f", a=1))
    bk_sb = const_p.tile([1, DPC], BF16)
    nc.gpsimd.dma_start(out=bk_sb, in_=bk.rearrange("(a f) -> a f", a=1))
    bv_sb = const_p.tile([1, DPC], BF16)
    nc.gpsimd.dma_start(out=bv_sb, in_=bv.rearrange("(a f) -> a f", a=1))

    # ---- staging casts (gpsimd DMA, fp32->bf16), priority order ----
    xt = xt_p.tile([128, FC, S], BF16)
    wvt = wvt_p.tile([128, FC, DPC], BF16)
    wqts = [wt_p.tile([128, FC, 128], BF16, tag="wt", name=f"wqt{g}")
            for g in range(NG)]
    wkts = [wt_p.tile([128, FC, 128], BF16, tag="wt", name=f"wkt{g}")
            for g in range(NG)]

    # x and the early-needed weights (Wq0/Wk0/Wv) are staged DIRECTLY in
    # fp32 (read-only DMA, no cast round-trip -- the cast happens on the
    # PSUM->SBUF copy after the PE transpose). This nearly halves the
    # prologue's DMA bytes, which pace the whole ramp-up.
    x_nat = [xstage_p.tile([128, H], F32, tag="xs", name=f"xn{sc}")
             for sc in range(SC)]
    w_nat = {}
    for key, wsrc, gs in (("q", wq, [0]), ("k", wk, [0]),
                          ("v", wv, [0, 1, 2, 3])):
        for g in gs:
            nat = wstage_p.tile([128, H], F32, tag="ws", name=f"wn_{key}{g}")
            w_nat[(key, g)] = nat
    # bf16 staging for the XBAR-transposed late W groups
    for g in range(1, NG):
        for key in ("q", "k"):
            nat = wstgb_p.tile([128, H], BF16, tag="wb", name=f"wb_{key}{g}")
            w_nat[(key, g)] = nat

    def x_dma(sc):
        nc.gpsimd.dma_start(out=x_nat[sc], in_=x[sc * 128:(sc + 1) * 128, :])

    # upfront DMAs: wq0, wk0, x0-7, wv0-3 (fp32, priority order)
    nc.gpsimd.dma_start(out=w_nat[("q", 0)], in_=wq[0:128, :])
    nc.gpsimd.dma_start(out=w_nat[("k", 0)], in_=wk[0:128, :])
    for sc in range(4):
        x_dma(sc)
    for dc in range(4):
        nc.gpsimd.dma_start(out=w_nat[("v", dc)],
                            in_=wv[dc * 128:(dc + 1) * 128, :])
    for sc in range(4, 8):
        x_dma(sc)
    # x8-15 DMAs are emitted as buffers free (after tp_x(sc-8));
    # Wq/Wk g1-3 casts + XBAR transposes are emitted from inside the
    # attention loop so they never compete with the ramp-up DMA.

    def emit_wg_late(g):
        for key, wsrc in (("q", wq), ("k", wk)):
            nat = w_nat[(key, g)]
            nc.gpsimd.dma_start(out=nat, in_=wsrc[g * 128:(g + 1) * 128, :])
        for key, dst in (("q", wqts[g]), ("k", wkts[g])):
            nat = w_nat[(key, g)]
            for fc in range(FC):
                nc.sync.dma_start_transpose(dst[:, fc, :],
                                            nat[:, fc * 128:(fc + 1) * 128])

    # ---- PE transpose packs (fp32 in, bf16 out via the copy) ----
    def tp_pack(dst_view, src_nat, fc0, pool, tag):
        pt = pool.tile([128, 4, 128], F32, tag=tag, name="pt")
        for j in range(4):
            fc = fc0 + j
            nc.tensor.transpose(pt[:, j, :],
                                src_nat[:, fc * 128:(fc + 1) * 128], ident)
            yield
        nc.vector.tensor_copy(dst_view, pt)
        yield

    def tp_x(sc, pool, tag):
        for fc0 in (0, 4):
            yield from tp_pack(xt[:, fc0:fc0 + 4, sc * 128:(sc + 1) * 128],
                               x_nat[sc], fc0, pool, tag)
        if sc + 8 < SC:
            x_dma(sc + 8)

    def tp_wqk(key, g, pool, tag):
        dst = wqts[g] if key == "q" else wkts[g]
        for fc0 in (0, 4):
            yield from tp_pack(dst[:, fc0:fc0 + 4, :], w_nat[(key, g)],
                               fc0, pool, tag)

    def tp_wv(dc, pool, tag):
        for fc0 in (0, 4):
            yield from tp_pack(wvt[:, fc0:fc0 + 4, dc * 128:(dc + 1) * 128],
                               w_nat[("v", dc)], fc0, pool, tag)

    # ---- projection chains ----
    # v_sb layout: [:, h, 0] = ones (denominator row), [:, h, 1:65] = V
    v_sb = [v_p.tile([128, HPC, DH + 1], BF16, tag="v", name=f"v{sc}")
            for sc in range(SC)]
    qts = {}
    kts = {}

    def get_qkt(kind, g):
        d = qts if kind == "qt" else kts
        if g not in d:
            d[g] = qkt_p.tile([128, S], BF16, tag="qkt", name=f"{kind}{g}")
        return d[g]

    def v_chain(sc, pool, tag):
        vp = pool.tile([128, DPC], F32, tag=tag, name=f"vp{sc}")
        for fc in range(FC):
            nc.tensor.matmul(vp, xt[:, fc, sc * 128:(sc + 1) * 128],
                             wvt[:, fc, :], start=(fc == 0), stop=False)
            yield
        nc.tensor.matmul(vp, ones512[:, 0:128], bv_sb, start=False, stop=True)
        nc.gpsimd.tensor_copy(v_sb[sc][:, :, 0:1], ones_col_f)
        yield
        nc.vector.tensor_copy(v_sb[sc][:, :, 1:DH + 1],
                              vp.rearrange("p (h d) -> p h d", d=DH))
        yield

    def qk_chain(kind, g, sg, pool, tag):
        bias_sb = bq_sb if kind == "qt" else bk_sb
        wt_src = wqts[g] if kind == "qt" else wkts[g]
        qk_dst = get_qkt(kind, g)
        ssl = slice(sg * 512, (sg + 1) * 512)
        qp = pool.tile([128, 512], F32, tag=tag, name=f"{kind}{g}s{sg}p")
        for fc in range(FC):
            nc.tensor.matmul(qp, wt_src[:, fc, :], xt[:, fc, ssl],
                             start=(fc == 0), stop=False)
            yield
        nc.tensor.matmul(qp, bias_sb[:, g * 128:(g + 1) * 128],
                         ones512, start=False, stop=True)
        yield
        nc.vector.tensor_copy(qk_dst[:, ssl], qp)
        yield

    done = set()

    def run_now(gen_):
        for _ in gen_:
            pass

    # ---- prologue: minimum to start the exp stream ----
    run_now(tp_wqk("q", 0, psum_ctx, "ctx"))
    run_now(tp_wqk("k", 0, psum_ctx, "ctx"))
    for sc in range(4):
        run_now(tp_x(sc, psum_ctx, "ctx"))
    run_now(qk_chain("kt", 0, 0, psum_ctx, "ctx"))
    run_now(qk_chain("qt", 0, 0, psum_ctx, "ctx"))
    done.update({"kt0s0", "qt0s0"})

    # ---- generators: A gates scores (kt/qt), B gates PV (v) ----
    plan_a = [("x", 4), ("x", 5), ("x", 6), ("x", 7), ("kt", 0, 1),
              ("x", 8), ("x", 9), ("x", 10), ("x", 11), ("kt", 0, 2),
              ("x", 12), ("x", 13), ("x", 14), ("x", 15), ("kt", 0, 3),
              ("qt", 0, 1), ("qt", 0, 2), ("qt", 0, 3)]
    for g in range(1, NG):
        plan_a += [("wgate", g), ("kt", g, 0), ("qt", g, 0), ("kt", g, 1),
                   ("kt", g, 2), ("kt", g, 3), ("qt", g, 1), ("qt", g, 2),
                   ("qt", g, 3)]
    wg_emitted = set()
    plan_b = ([("wv", dc) for dc in range(4)] +
              [("v", sc) for sc in range(SC)])

    def run_plan(plan, pool, tag):
        for item in plan:
            if item[0] == "wgate":
                # stall (empty yields) until the flat loop has emitted the
                # casts + XBAR transposes for this W group
                while item[1] not in wg_emitted:
                    yield
            elif item[0] == "x":
                yield from tp_x(item[1], pool, tag)
                done.add(f"x{item[1]}")
            elif item[0] == "wv":
                yield from tp_wv(item[1], pool, tag)
            elif item[0] == "v":
                sc = item[1]
                if sc >= 4:
                    # xt[sc] comes from gen-A: force its pack to be
                    # emitted first (cross-generator RAW dependency)
                    need(0, f"x{sc}")
                yield from v_chain(sc, pool, tag)
                done.add(f"v{sc}")
            else:
                kind, g, sg = item
                yield from qk_chain(kind, g, sg, pool, tag)
                done.add(f"{kind}{g}s{sg}")

    gens = [run_plan(plan_a, psum_qa, "qa"), run_plan(plan_b, psum_qb, "qb")]

    def drive(n):
        # round-robin both generators
        for _ in range(n):
            alive = [g for g in gens if g is not None]
            if not alive:
                return
            for idx in range(2):
                if gens[idx] is None:
                    continue
                try:
                    next(gens[idx])
                except StopIteration:
                    gens[idx] = None

    def need(idx, *products):
        while gens[idx] is not None and not all(p in done for p in products):
            try:
                next(gens[idx])
            except StopIteration:
                gens[idx] = None

    # ---- attention: flat pipelined stream ----
    blocks = [(g2, qg) for g2 in range(NG) for qg in range(QG)]
    T = len(blocks) * KC
    pend_st = {}
    cur_cp = {}

    def emit_scores(t):
        bi, kc = divmod(t, KC)
        g2, qg = blocks[bi]
        qt, kt = get_qkt("qt", g2), get_qkt("kt", g2)
        ksl = slice(kc * 128, (kc + 1) * 128)
        qsl = slice(qg * 512, (qg + 1) * 512)
        st = psum_mm.tile([128, 2, 512], F32, tag="mm")
        nc.tensor.matmul(st[:, 0, :], kt[0:64, ksl], qt[0:64, qsl],
                         start=True, stop=True)
        nc.tensor.matmul(st[:, 1, :], kt[64:128, ksl], qt[64:128, qsl],
                         start=True, stop=True)
        pend_st[t] = st

    pend_ex = {}
    post = []   # deferred epilogue closures (recip/bcast/mul/dma)

    if dbg_aps:
        dbg2_p = ctx.enter_context(tc.tile_pool(name="dbgt", bufs=1))
        dbg_ex = dbg2_p.tile([128, 2, 512], BF16, tag="dx", name="dbg_ex")
        dbg_ctxs = dbg2_p.tile([DH + 1, 512], F32, tag="dc", name="dbg_ctxs")
        dbg_rec = dbg2_p.tile([1, 512], F32, tag="dr", name="dbg_rec")
        dbg_bc = dbg2_p.tile([DH + 1, 512], F32, tag="db", name="dbg_bc")

    def epilogue_a(bi, cp0, cp1):
        g2, qg = blocks[bi]
        qsl = slice(qg * 512, (qg + 1) * 512)
        for h_loc, cp in ((0, cp0), (1, cp1)):
            h = 2 * g2 + h_loc
            ctxs = ctxs_p.tile([DH + 1, 512], F32, tag="ctxs")
            nc.vector.tensor_copy(ctxs, cp)
            first = (bi == 0 and h_loc == 0)
            if dbg_aps and first:
                nc.vector.tensor_copy(dbg_ctxs, ctxs)

            def fin(h=h, ctxs=ctxs, qsl=qsl, first=first):
                rec = rec_p.tile([1, 512], F32, tag="rec")
                nc.vector.reciprocal_approx_fast(rec, ctxs[0:1, :])
                bc = bcs_p.tile([DH + 1, 512], F32, tag="bc")
                nc.gpsimd.partition_broadcast(bc, rec)
                ot = outt_p.tile([DH + 1, 512], F32, tag="outt")
                nc.vector.tensor_mul(ot, ctxs, bc)
                nc.sync.dma_start(out=outT[h * DH:(h + 1) * DH, qsl],
                                  in_=ot[1:DH + 1, :])
                if dbg_aps and first:
                    nc.vector.tensor_copy(dbg_rec, rec)
                    nc.vector.tensor_copy(dbg_bc, bc)

            post.append(fin)

    for k in range(AHEAD):
        emit_scores(k)
    drive(12)
    for t in range(T + LAG):
        if t == 28:
            emit_wg_late(1)
            wg_emitted.add(1)
        elif t == 64:
            emit_wg_late(2)
            wg_emitted.add(2)
        elif t == 100:
            emit_wg_late(3)
            wg_emitted.add(3)
        if t < T:
            bi, kc = divmod(t, KC)
            st = pend_st.pop(t)
            ex = ex_p.tile([128, 2, 512], BF16, tag="ex")
            nc.scalar.activation(ex.rearrange("p a b -> p (a b)"),
                                 st.rearrange("p a b -> p (a b)"),
                                 AF.Exp, bias=mask_sb[:, kc:kc + 1],
                                 scale=1.0 / np.sqrt(DH))
            pend_ex[t] = ex
            if dbg_aps and t == 0:
                nc.gpsimd.tensor_copy(dbg_ex, ex)
            if t + AHEAD < T:
                nbi, nkc = divmod(t + AHEAD, KC)
                ng2, nqg = blocks[nbi]
                need(0, f"kt{ng2}s{nkc // 4}", f"qt{ng2}s{nqg}")
                emit_scores(t + AHEAD)
        pt_ = t - LAG
        if pt_ >= 0:
            pbi, pkc = divmod(pt_, KC)
            pg2, _ = blocks[pbi]
            if pkc == 0:
                cpa = psum_ctx.tile([DH + 1, 512], F32, tag="ctx", name="cpa")
                cpb = psum_ctx.tile([DH + 1, 512], F32, tag="ctx", name="cpb")
                cur_cp[pbi] = (cpa, cpb)
            cp0, cp1 = cur_cp[pbi]
            if pbi == 0:
                need(1, f"v{pkc}")
            ex = pend_ex.pop(pt_)
            nc.tensor.matmul(cp0, v_sb[pkc][:, 2 * pg2, :], ex[:, 0, :],
                             start=(pkc == 0), stop=(pkc == KC - 1))
            nc.tensor.matmul(cp1, v_sb[pkc][:, 2 * pg2 + 1, :], ex[:, 1, :],
                             start=(pkc == 0), stop=(pkc == KC - 1))
            if pkc == KC - 1:
                epilogue_a(pbi, cp0, cp1)
                del cur_cp[pbi]
        if (t % KC) in (9, 13) and post:
            post.pop(0)()
        drive(2)

    while post:
        post.pop(0)()
    for idx in range(2):
        while gens[idx] is not None:
            try:
                next(gens[idx])
            except StopIteration:
                gens[idx] = None

    if dbg_aps:
        nc.sync.dma_start(out=dbg_aps["xt"], in_=xt)
        nc.sync.dma_start(out=dbg_aps["qt0"], in_=qts[0])
        nc.sync.dma_start(out=dbg_aps["kt0"], in_=kts[0])
        nc.sync.dma_start(out=dbg_aps["v0"], in_=v_sb[0])
        nc.sync.dma_start(out=dbg_aps["ex00"], in_=dbg_ex)
        nc.sync.dma_start(out=dbg_aps["ctxs0"], in_=dbg_ctxs)
        nc.sync.dma_start(out=dbg_aps["rec0"], in_=dbg_rec)
        nc.sync.dma_start(out=dbg_aps["bc0"], in_=dbg_bc)


def build_program():
    nc = bacc.Bacc("TRN2", target_bir_lowering=False, debug=False)
    x = nc.dram_tensor("x", [S, H], F32, kind="ExternalInput").ap()
    mask = nc.dram_tensor("mask", [S], F32, kind="ExternalInput").ap()
    wq = nc.dram_tensor("wq", [DPC, H], F32, kind="ExternalInput").ap()
    bq = nc.dram_tensor("bq", [DPC], F32, kind="ExternalInput").ap()
    wk = nc.dram_tensor("wk", [DPC, H], F32, kind="ExternalInput").ap()
    bk = nc.dram_tensor("bk", [DPC], F32, kind="ExternalInput").ap()
    wv = nc.dram_tensor("wv", [DPC, H], F32, kind="ExternalInput").ap()
    bv = nc.dram_tensor("bv", [DPC], F32, kind="ExternalInput").ap()
    outT = nc.dram_tensor("outT", [DPC, S], F32, kind="ExternalOutput").ap()

    import os
    if os.environ.get("K_DEBUG"):
        nc._dbg_aps = {
            "xt": nc.dram_tensor("xt_dbg", [128, FC, S], BF16,
                                 kind="ExternalOutput").ap(),
            "qt0": nc.dram_tensor("qt0_dbg", [128, S], BF16,
                                  kind="ExternalOutput").ap(),
            "kt0": nc.dram_tensor("kt0_dbg", [128, S], BF16,
                                  kind="ExternalOutput").ap(),
            "v0": nc.dram_tensor("v0_dbg", [128, HPC, DH + 1], BF16,
                                 kind="ExternalOutput").ap(),
            "ex00": nc.dram_tensor("ex00_dbg", [128, 2, 512], BF16,
                                   kind="ExternalOutput").ap(),
            "ctxs0": nc.dram_tensor("ctxs0_dbg", [DH + 1, 512], F32,
                                    kind="ExternalOutput").ap(),
            "rec0": nc.dram_tensor("rec0_dbg", [1, 512], F32,
                                   kind="ExternalOutput").ap(),
            "bc0": nc.dram_tensor("bc0_dbg", [DH + 1, 512], F32,
                                  kind="ExternalOutput").ap(),
        }

    from contextlib import ExitStack
    with tile.TileContext(nc) as tc:
        with ExitStack() as ctx:
            _emit(ctx, tc, nc, x, mask, wq, bq, wk, bk, wv, bv, outT)
    nc.compile()
    return nc


_NC_CACHE = None


def make_in_maps(hidden_states, attention_mask, Wq, bq, Wk, bk, Wv, bv):
    hs = np.asarray(hidden_states, dtype=np.float32)
    am = np.asarray(attention_mask, dtype=np.float32)
    ws = {k: np.asarray(v, dtype=np.float32)
          for k, v in (("wq", Wq), ("bq", bq), ("wk", Wk),
                       ("bk", bk), ("wv", Wv), ("bv", bv))}
    in_maps = []
    for c in range(N_CORES):
        b, g = divmod(c, 2)
        sl = slice(g * DPC, (g + 1) * DPC)
        in_maps.append({
            "x": np.ascontiguousarray(hs[:, b, :]),
            "mask": np.ascontiguousarray(am[b, 0, 0, :]),
            "wq": np.ascontiguousarray(ws["wq"][sl]),
            "bq": np.ascontiguousarray(ws["bq"][sl]),
            "wk": np.ascontiguousarray(ws["wk"][sl]),
            "bk": np.ascontiguousarray(ws["bk"][sl]),
            "wv": np.ascontiguousarray(ws["wv"][sl]),
            "bv": np.ascontiguousarray(ws["bv"][sl]),
        })
    return in_maps


def gather_out(results):
    out = np.empty((S, B, H), np.float32)
    for c in range(N_CORES):
        b, g = divmod(c, 2)
        out[:, b, g * DPC:(g + 1) * DPC] = results[c]["outT"].T
    return out


def kernel(hidden_states, attention_mask, Wq, bq, Wk, bk, Wv, bv):
    global _NC_CACHE
    if _NC_CACHE is None:
        _NC_CACHE = build_program()
    in_maps = make_in_maps(hidden_states, attention_mask,
                           Wq, bq, Wk, bk, Wv, bv)
    res = run_bass_kernel_spmd(_NC_CACHE, in_maps, list(range(N_CORES)))
    return gather_out(res.results)


# revision 32
# speedup vs baseline: 1.2593x; 1.0340x over previous
"""BertSelfAttention Trainium2 Bass kernel (v4: fully-overlapped pipeline).

Problem: S=2048, B=4, H=1024, NH=16, DH=64, fp32.
  q/k/v = hidden @ W{q,k,v}.T + b   -> softmax((q k^T)/8 + mask) @ v

Sharding over 8 cores: batch (4) x head-group (2 groups of 8 heads).
Each core gets x=[2048,1024] (its batch), W shards [512,1024] (its 8
heads), mask [2048], and produces outT=[512,2048] (feature-major) which
the host transposes and scatters into the full [S,B,H] output.

The kernel is exp-bound: 256 ScalarE activations of [128,1024] at
~1.34us each are the hard floor. v4 hides everything else behind that
stream:
  - x/W are cast fp32->bf16 by gpsimd DMAs; x and group-0 W transposes
    run as PE 4-block packs (bf16, 1 cyc/row); Wq/Wk groups 1-3 go
    through the XBAR dma_start_transpose (measured ~26GB/s serial on
    HW, fine for late-needed weights, frees the PE)
  - a flat 256-iteration stream emits ACT(t), scores(t+1), PV(t-4):
    the 4-tile PV lag (deep ex buffering) lets the exp stream run ahead
    while V projections are still being produced in block 0
  - two independent generators interleave production into the loop:
    gen-A (x transpose packs + K/Q chains, gating scores) and gen-B
    (Wv packs + V chains, gating only PV), each with its own PSUM bank,
    pulled by need() milestones so kt pulls never drag V work in early
  - PV accumulates [1+64, 512] per head with a leading ones-row (the
    softmax denominator lands in PSUM partition 0 for free); the
    epilogue inverts that row in place (RECIPROCAL_APPROX_FAST on
    [1,512], all partition offsets 0 -- offset-mismatched DVE operands
    return garbage on HW), broadcasts it across partitions with
    gpsimd.partition_broadcast, multiplies on DVE, and DMAs the [d, q]
    tile out feature-major on the sync queue; the host transposes
    during gather (off-device). The recip/bcast/mul/DMA part is
    deferred a few iterations so the PE never stalls at boundaries.
"""

import numpy as np

import concourse.bass as bass
import concourse.mybir as mybir
import concourse.tile as tile
from concourse import bacc
from concourse.bass_utils import run_bass_kernel_spmd
from concourse.masks import make_identity

F32 = mybir.dt.float32
BF16 = mybir.dt.bfloat16
AF = mybir.ActivationFunctionType

S, B, H, NH, DH = 2048, 4, 1024, 16, 64
N_CORES = 8
HPC = 8            # heads per core
DPC = HPC * DH     # 512 output features per core
SC = S // 128      # 16 s-chunks
FC = H // 128      # 8 feature chunks
QG = S // 512      # 4 query groups
KC = S // 128      # 16 key chunks
NG = 4             # head-pair groups per core
LAG = 8            # PV trails ACT by this many tiles
AHEAD = 4          # scores are emitted this many tiles ahead of ACT


def _emit(ctx, tc, nc, x, mask, wq, bq, wk, bk, wv, bv, outT):
    import os
    dbg_aps = getattr(nc, "_dbg_aps", None) if os.environ.get("K_DEBUG") else None

    const_p = ctx.enter_context(tc.tile_pool(name="const", bufs=1))
    xstage_p = ctx.enter_context(tc.tile_pool(name="xstage", bufs=8))
    wstage_p = ctx.enter_context(tc.tile_pool(name="wstage", bufs=6))
    wstgb_p = ctx.enter_context(tc.tile_pool(name="wstgb", bufs=6))
    xt_p = ctx.enter_context(tc.tile_pool(name="xt", bufs=1))
    wvt_p = ctx.enter_context(tc.tile_pool(name="wvt", bufs=1))
    wt_p = ctx.enter_context(tc.tile_pool(name="wt", bufs=8))
    v_p = ctx.enter_context(tc.tile_pool(name="v", bufs=SC))
    qkt_p = ctx.enter_context(tc.tile_pool(name="qkt", bufs=4))
    ex_p = ctx.enter_context(tc.tile_pool(name="ex", bufs=12))
    ctxs_p = ctx.enter_context(tc.tile_pool(name="ctxs", bufs=2))
    rec_p = ctx.enter_context(tc.tile_pool(name="rec", bufs=2))
    bcs_p = ctx.enter_context(tc.tile_pool(name="bcs", bufs=2))
    outt_p = ctx.enter_context(tc.tile_pool(name="outt", bufs=4))

    # psum (8 banks): mm 2x2 (score tiles) + ctx 2x1 (PV accumulators /
    # prologue packs+chains) + qa 1 (gen-A) + qb 1 (gen-B)
    psum_mm = ctx.enter_context(tc.tile_pool(name="psmm", bufs=2, space="PSUM"))
    psum_ctx = ctx.enter_context(tc.tile_pool(name="psctx", bufs=2, space="PSUM"))
    psum_qa = ctx.enter_context(tc.tile_pool(name="psqa", bufs=1, space="PSUM"))
    psum_qb = ctx.enter_context(tc.tile_pool(name="psqb", bufs=1, space="PSUM"))

    # ---- constants ----
    mask_sb = const_p.tile([128, KC], F32)
    nc.sync.dma_start(out=mask_sb, in_=mask.rearrange("(c p) -> p c", p=128))

    ident = const_p.tile([128, 128], F32)
    make_identity(nc, ident)

    ones_col_f = const_p.tile([128, HPC, 1], F32)
    nc.vector.memset(ones_col_f, 1.0)
    # q/k biases as per-partition scalars [128, g] (features on partitions);
    # added during the chain's PSUM->SBUF copy, not on the PE
    bq_sb = const_p.tile([128, NG], F32)
    nc.sync.dma_start(out=bq_sb, in_=bq.rearrange("(g p) -> p g", p=128))
    bk_sb = const_p.tile([128, NG], F32)
    nc.sync.dma_start(out=bk_sb, in_=bk.rearrange("(g p) -> p g", p=128))
    # v bias broadcast across partitions (sequence on partitions)
    bv_row = const_p.tile([1, DPC], F32)
    nc.sync.dma_start(out=bv_row, in_=bv.rearrange("(a f) -> a f", a=1))
    bv_bc = const_p.tile([128, DPC], F32)
    nc.gpsimd.partition_broadcast(bv_bc, bv_row)

    # ---- staging casts (gpsimd DMA, fp32->bf16), priority order ----
    xt = xt_p.tile([128, FC, S], BF16)
    wvt = wvt_p.tile([128, FC, DPC], BF16)
    wqts = [wt_p.tile([128, FC, 128], BF16, tag="wt", name=f"wqt{g}")
            for g in range(NG)]
    wkts = [wt_p.tile([128, FC, 128], BF16, tag="wt", name=f"wkt{g}")
            for g in range(NG)]

    # x and the early-needed weights (Wq0/Wk0/Wv) are staged DIRECTLY in
    # fp32 (read-only DMA, no cast round-trip -- the cast happens on the
    # PSUM->SBUF copy after the PE transpose). This nearly halves the
    # prologue's DMA bytes, which pace the whole ramp-up.
    x_nat = [xstage_p.tile([128, H], F32, tag="xs", name=f"xn{sc}")
             for sc in range(SC)]
    w_nat = {}
    for key, wsrc, gs in (("q", wq, [0]), ("k", wk, [0]),
                          ("v", wv, [0, 1, 2, 3])):
        for g in gs:
            nat = wstage_p.tile([128, H], F32, tag="ws", name=f"wn_{key}{g}")
            w_nat[(key, g)] = nat
    # bf16 staging for the XBAR-transposed late W groups
    for g in range(1, NG):
        for key in ("q", "k"):
            nat = wstgb_p.tile([128, H], BF16, tag="wb", name=f"wb_{key}{g}")
            w_nat[(key, g)] = nat

    def x_dma(sc):
        nc.gpsimd.dma_start(out=x_nat[sc], in_=x[sc * 128:(sc + 1) * 128, :])

    # upfront DMAs: wq0, wk0, x0-7, wv0-3 (fp32, priority order)
    nc.gpsimd.dma_start(out=w_nat[("q", 0)], in_=wq[0:128, :])
    nc.gpsimd.dma_start(out=w_nat[("k", 0)], in_=wk[0:128, :])
    for sc in range(4):
        x_dma(sc)
    for dc in range(4):
        nc.gpsimd.dma_start(out=w_nat[("v", dc)],
                            in_=wv[dc * 128:(dc + 1) * 128, :])
    for sc in range(4, 8):
        x_dma(sc)
    # x8-15 DMAs are emitted as buffers free (after tp_x(sc-8));
    # Wq/Wk g1-3 casts + XBAR transposes are emitted from inside the
    # attention loop so they never compete with the ramp-up DMA.

    def emit_wg_late(g):
        for key, wsrc in (("q", wq), ("k", wk)):
            nat = w_nat[(key, g)]
            nc.gpsimd.dma_start(out=nat, in_=wsrc[g * 128:(g + 1) * 128, :])
        for key, dst in (("q", wqts[g]), ("k", wkts[g])):
            nat = w_nat[(key, g)]
            for fc in range(FC):
                nc.sync.dma_start_transpose(dst[:, fc, :],
                                            nat[:, fc * 128:(fc + 1) * 128])

    # ---- PE transpose packs (fp32 in, bf16 out via the copy) ----
    def tp_pack(dst_view, src_nat, fc0, pool, tag):
        pt = pool.tile([128, 4, 128], F32, tag=tag, name="pt")
        for j in range(4):
            fc = fc0 + j
            nc.tensor.transpose(pt[:, j, :],
                                src_nat[:, fc * 128:(fc + 1) * 128], ident)
            yield
        nc.vector.tensor_copy(dst_view, pt)
        yield

    def tp_x(sc, pool, tag):
        for fc0 in (0, 4):
            yield from tp_pack(xt[:, fc0:fc0 + 4, sc * 128:(sc + 1) * 128],
                               x_nat[sc], fc0, pool, tag)
        if sc + 8 < SC:
            x_dma(sc + 8)

    def tp_wqk(key, g, pool, tag):
        dst = wqts[g] if key == "q" else wkts[g]
        for fc0 in (0, 4):
            yield from tp_pack(dst[:, fc0:fc0 + 4, :], w_nat[(key, g)],
                               fc0, pool, tag)

    def tp_wv(dc, pool, tag):
        for fc0 in (0, 4):
            yield from tp_pack(wvt[:, fc0:fc0 + 4, dc * 128:(dc + 1) * 128],
                               w_nat[("v", dc)], fc0, pool, tag)

    # ---- projection chains ----
    # v_sb layout: [:, h, 0] = ones (denominator row), [:, h, 1:65] = V
    v_sb = [v_p.tile([128, HPC, DH + 1], BF16, tag="v", name=f"v{sc}")
            for sc in range(SC)]
    qts = {}
    kts = {}

    def get_qkt(kind, g):
        d = qts if kind == "qt" else kts
        if g not in d:
            d[g] = qkt_p.tile([128, S], BF16, tag="qkt", name=f"{kind}{g}")
        return d[g]

    def v_chain(sc, pool, tag):
        vp = pool.tile([128, DPC], F32, tag=tag, name=f"vp{sc}")
        for fc in range(FC):
            nc.tensor.matmul(vp, xt[:, fc, sc * 128:(sc + 1) * 128],
                             wvt[:, fc, :], start=(fc == 0),
                             stop=(fc == FC - 1))
            yield
        nc.gpsimd.tensor_copy(v_sb[sc][:, :, 0:1], ones_col_f)
        nc.vector.tensor_add(v_sb[sc][:, :, 1:DH + 1],
                             vp.rearrange("p (h d) -> p h d", d=DH),
                             bv_bc.rearrange("p (h d) -> p h d", d=DH))
        yield

    def qk_chain(kind, g, sg, pool, tag):
        bias_sb = bq_sb if kind == "qt" else bk_sb
        wt_src = wqts[g] if kind == "qt" else wkts[g]
        qk_dst = get_qkt(kind, g)
        ssl = slice(sg * 512, (sg + 1) * 512)
        qp = pool.tile([128, 512], F32, tag=tag, name=f"{kind}{g}s{sg}p")
        for fc in range(FC):
            nc.tensor.matmul(qp, wt_src[:, fc, :], xt[:, fc, ssl],
                             start=(fc == 0), stop=(fc == FC - 1))
            yield
        nc.vector.tensor_scalar_add(qk_dst[:, ssl], qp, bias_sb[:, g:g + 1])
        yield

    done = set()

    def run_now(gen_):
        for _ in gen_:
            pass

    # ---- prologue: minimum to start the exp stream ----
    run_now(tp_wqk("q", 0, psum_ctx, "ctx"))
    run_now(tp_wqk("k", 0, psum_ctx, "ctx"))
    for sc in range(4):
        run_now(tp_x(sc, psum_ctx, "ctx"))
    run_now(qk_chain("kt", 0, 0, psum_ctx, "ctx"))
    run_now(qk_chain("qt", 0, 0, psum_ctx, "ctx"))
    done.update({"kt0s0", "qt0s0"})

    # ---- generators: A gates scores (kt/qt), B gates PV (v) ----
    plan_a = [("x", 4), ("x", 5), ("x", 6), ("x", 7), ("kt", 0, 1),
              ("x", 8), ("x", 9), ("x", 10), ("x", 11), ("kt", 0, 2),
              ("x", 12), ("x", 13), ("x", 14), ("x", 15), ("kt", 0, 3),
              ("qt", 0, 1), ("qt", 0, 2), ("qt", 0, 3)]
    for g in range(1, NG):
        plan_a += [("wgate", g), ("kt", g, 0), ("qt", g, 0), ("kt", g, 1),
                   ("kt", g, 2), ("kt", g, 3), ("qt", g, 1), ("qt", g, 2),
                   ("qt", g, 3)]
    wg_emitted = set()
    plan_b = ([("wv", dc) for dc in range(4)] +
              [("v", sc) for sc in range(SC)])

    def run_plan(plan, pool, tag):
        for item in plan:
            if item[0] == "wgate":
                # stall (empty yields) until the flat loop has emitted the
                # casts + XBAR transposes for this W group
                while item[1] not in wg_emitted:
                    yield
            elif item[0] == "x":
                yield from tp_x(item[1], pool, tag)
                done.add(f"x{item[1]}")
            elif item[0] == "wv":
                yield from tp_wv(item[1], pool, tag)
            elif item[0] == "v":
                sc = item[1]
                if sc >= 4:
                    # xt[sc] comes from gen-A: force its pack to be
                    # emitted first (cross-generator RAW dependency)
                    need(0, f"x{sc}")
                yield from v_chain(sc, pool, tag)
                done.add(f"v{sc}")
            else:
                kind, g, sg = item
                yield from qk_chain(kind, g, sg, pool, tag)
                done.add(f"{kind}{g}s{sg}")

    gens = [run_plan(plan_a, psum_qa, "qa"), run_plan(plan_b, psum_qb, "qb")]

    def drive(n):
        # round-robin both generators
        for _ in range(n):
            alive = [g for g in gens if g is not None]
            if not alive:
                return
            for idx in range(2):
                if gens[idx] is None:
                    continue
                try:
                    next(gens[idx])
                except StopIteration:
                    gens[idx] = None

    def need(idx, *products):
        while gens[idx] is not None and not all(p in done for p in products):
            try:
                next(gens[idx])
            except StopIteration:
                gens[idx] = None

    # ---- attention: flat pipelined stream ----
    blocks = [(g2, qg) for g2 in range(NG) for qg in range(QG)]
    T = len(blocks) * KC
    pend_st = {}
    cur_cp = {}

    def emit_scores(t):
        bi, kc = divmod(t, KC)
        g2, qg = blocks[bi]
        qt, kt = get_qkt("qt", g2), get_qkt("kt", g2)
        ksl = slice(kc * 128, (kc + 1) * 128)
        qsl = slice(qg * 512, (qg + 1) * 512)
        st = psum_mm.tile([128, 2, 512], F32, tag="mm")
        nc.tensor.matmul(st[:, 0, :], kt[0:64, ksl], qt[0:64, qsl],
                         start=True, stop=True)
        nc.tensor.matmul(st[:, 1, :], kt[64:128, ksl], qt[64:128, qsl],
                         start=True, stop=True)
        pend_st[t] = st

    pend_ex = {}
    post = []   # deferred epilogue closures (recip/bcast/mul/dma)

    if dbg_aps:
        dbg2_p = ctx.enter_context(tc.tile_pool(name="dbgt", bufs=1))
        dbg_ex = dbg2_p.tile([128, 2, 512], BF16, tag="dx", name="dbg_ex")
        dbg_ctxs = dbg2_p.tile([DH + 1, 512], F32, tag="dc", name="dbg_ctxs")
        dbg_rec = dbg2_p.tile([1, 512], F32, tag="dr", name="dbg_rec")
        dbg_bc = dbg2_p.tile([DH + 1, 512], F32, tag="db", name="dbg_bc")

    def epilogue_a(bi, cp0, cp1):
        g2, qg = blocks[bi]
        qsl = slice(qg * 512, (qg + 1) * 512)
        for h_loc, cp in ((0, cp0), (1, cp1)):
            h = 2 * g2 + h_loc
            ctxs = ctxs_p.tile([DH + 1, 512], F32, tag="ctxs")
            nc.vector.tensor_copy(ctxs, cp)
            first = (bi == 0 and h_loc == 0)
            if dbg_aps and first:
                nc.vector.tensor_copy(dbg_ctxs, ctxs)

            def fin(h=h, ctxs=ctxs, qsl=qsl, first=first):
                rec = rec_p.tile([1, 512], F32, tag="rec")
                nc.vector.reciprocal_approx_fast(rec, ctxs[0:1, :])
                bc = bcs_p.tile([DH + 1, 512], F32, tag="bc")
                nc.gpsimd.partition_broadcast(bc, rec)
                ot = outt_p.tile([DH + 1, 512], F32, tag="outt")
                nc.vector.tensor_mul(ot, ctxs, bc)
                nc.sync.dma_start(out=outT[h * DH:(h + 1) * DH, qsl],
                                  in_=ot[1:DH + 1, :])
                if dbg_aps and first:
                    nc.vector.tensor_copy(dbg_rec, rec)
                    nc.vector.tensor_copy(dbg_bc, bc)

            post.append(fin)

    for k in range(AHEAD):
        emit_scores(k)
    drive(12)
    for t in range(T + LAG):
        if t == 28:
            emit_wg_late(1)
            wg_emitted.add(1)
        elif t == 64:
            emit_wg_late(2)
            wg_emitted.add(2)
        elif t == 100:
            emit_wg_late(3)
            wg_emitted.add(3)
        if t < T:
            bi, kc = divmod(t, KC)
            st = pend_st.pop(t)
            ex = ex_p.tile([128, 2, 512], BF16, tag="ex")
            nc.scalar.activation(ex.rearrange("p a b -> p (a b)"),
                                 st.rearrange("p a b -> p (a b)"),
                                 AF.Exp, bias=mask_sb[:, kc:kc + 1],
                                 scale=1.0 / np.sqrt(DH))
            pend_ex[t] = ex
            if dbg_aps and t == 0:
                nc.gpsimd.tensor_copy(dbg_ex, ex)
            if t + AHEAD < T:
                nbi, nkc = divmod(t + AHEAD, KC)
                ng2, nqg = blocks[nbi]
                need(0, f"kt{ng2}s{nkc // 4}", f"qt{ng2}s{nqg}")
                emit_scores(t + AHEAD)
        pt_ = t - LAG
        if pt_ >= 0:
            pbi, pkc = divmod(pt_, KC)
            pg2, _ = blocks[pbi]
            if pkc == 0:
                cpa = psum_ctx.tile([DH + 1, 512], F32, tag="ctx", name="cpa")
                cpb = psum_ctx.tile([DH + 1, 512], F32, tag="ctx", name="cpb")
                cur_cp[pbi] = (cpa, cpb)
            cp0, cp1 = cur_cp[pbi]
            if pbi == 0:
                need(1, f"v{pkc}")
            ex = pend_ex.pop(pt_)
            nc.tensor.matmul(cp0, v_sb[pkc][:, 2 * pg2, :], ex[:, 0, :],
                             start=(pkc == 0), stop=(pkc == KC - 1))
            nc.tensor.matmul(cp1, v_sb[pkc][:, 2 * pg2 + 1, :], ex[:, 1, :],
                             start=(pkc == 0), stop=(pkc == KC - 1))
            if pkc == KC - 1:
                epilogue_a(pbi, cp0, cp1)
                del cur_cp[pbi]
        if (t % KC) in (9, 13) and post:
            post.pop(0)()
        drive(2)

    while post:
        post.pop(0)()
    for idx in range(2):
        while gens[idx] is not None:
            try:
                next(gens[idx])
            except StopIteration:
                gens[idx] = None

    if dbg_aps:
        nc.sync.dma_start(out=dbg_aps["xt"], in_=xt)
        nc.sync.dma_start(out=dbg_aps["qt0"], in_=qts[0])
        nc.sync.dma_start(out=dbg_aps["kt0"], in_=kts[0])
        nc.sync.dma_start(out=dbg_aps["v0"], in_=v_sb[0])
        nc.sync.dma_start(out=dbg_aps["ex00"], in_=dbg_ex)
        nc.sync.dma_start(out=dbg_aps["ctxs0"], in_=dbg_ctxs)
        nc.sync.dma_start(out=dbg_aps["rec0"], in_=dbg_rec)
        nc.sync.dma_start(out=dbg_aps["bc0"], in_=dbg_bc)


def build_program():
    nc = bacc.Bacc("TRN2", target_bir_lowering=False, debug=False)
    x = nc.dram_tensor("x", [S, H], F32, kind="ExternalInput").ap()
    mask = nc.dram_tensor("mask", [S], F32, kind="ExternalInput").ap()
    wq = nc.dram_tensor("wq", [DPC, H], F32, kind="ExternalInput").ap()
    bq = nc.dram_tensor("bq", [DPC], F32, kind="ExternalInput").ap()
    wk = nc.dram_tensor("wk", [DPC, H], F32, kind="ExternalInput").ap()
    bk = nc.dram_tensor("bk", [DPC], F32, kind="ExternalInput").ap()
    wv = nc.dram_tensor("wv", [DPC, H], F32, kind="ExternalInput").ap()
    bv = nc.dram_tensor("bv", [DPC], F32, kind="ExternalInput").ap()
    outT = nc.dram_tensor("outT", [DPC, S], F32, kind="ExternalOutput").ap()

    import os
    if os.environ.get("K_DEBUG"):
        nc._dbg_aps = {
            "xt": nc.dram_tensor("xt_dbg", [128, FC, S], BF16,
                                 kind="ExternalOutput").ap(),
            "qt0": nc.dram_tensor("qt0_dbg", [128, S], BF16,
                                  kind="ExternalOutput").ap(),
            "kt0": nc.dram_tensor("kt0_dbg", [128, S], BF16,
                                  kind="ExternalOutput").ap(),
            "v0": nc.dram_tensor("v0_dbg", [128, HPC, DH + 1], BF16,
                                 kind="ExternalOutput").ap(),
            "ex00": nc.dram_tensor("ex00_dbg", [128, 2, 512], BF16,
                                   kind="ExternalOutput").ap(),
            "ctxs0": nc.dram_tensor("ctxs0_dbg", [DH + 1, 512], F32,
                                    kind="ExternalOutput").ap(),
            "rec0": nc.dram_tensor("rec0_dbg", [1, 512], F32,
                                   kind="ExternalOutput").ap(),
            "bc0": nc.dram_tensor("bc0_dbg", [DH + 1, 512], F32,
                                  kind="ExternalOutput").ap(),
        }

    from contextlib import ExitStack
    with tile.TileContext(nc) as tc:
        with ExitStack() as ctx:
            _emit(ctx, tc, nc, x, mask, wq, bq, wk, bk, wv, bv, outT)
    nc.compile()
    return nc


_NC_CACHE = None


def make_in_maps(hidden_states, attention_mask, Wq, bq, Wk, bk, Wv, bv):
    hs = np.asarray(hidden_states, dtype=np.float32)
    am = np.asarray(attention_mask, dtype=np.float32)
    ws = {k: np.asarray(v, dtype=np.float32)
          for k, v in (("wq", Wq), ("bq", bq), ("wk", Wk),
                       ("bk", bk), ("wv", Wv), ("bv", bv))}
    in_maps = []
    for c in range(N_CORES):
        b, g = divmod(c, 2)
        sl = slice(g * DPC, (g + 1) * DPC)
        in_maps.append({
            "x": np.ascontiguousarray(hs[:, b, :]),
            "mask": np.ascontiguousarray(am[b, 0, 0, :]),
            "wq": np.ascontiguousarray(ws["wq"][sl]),
            "bq": np.ascontiguousarray(ws["bq"][sl]),
            "wk": np.ascontiguousarray(ws["wk"][sl]),
            "bk": np.ascontiguousarray(ws["bk"][sl]),
            "wv": np.ascontiguousarray(ws["wv"][sl]),
            "bv": np.ascontiguousarray(ws["bv"][sl]),
        })
    return in_maps


def gather_out(results):
    out = np.empty((S, B, H), np.float32)
    for c in range(N_CORES):
        b, g = divmod(c, 2)
        out[:, b, g * DPC:(g + 1) * DPC] = results[c]["outT"].T
    return out


def kernel(hidden_states, attention_mask, Wq, bq, Wk, bk, Wv, bv):
    global _NC_CACHE
    if _NC_CACHE is None:
        _NC_CACHE = build_program()
    in_maps = make_in_maps(hidden_states, attention_mask,
                           Wq, bq, Wk, bk, Wv, bv)
    res = run_bass_kernel_spmd(_NC_CACHE, in_maps, list(range(N_CORES)))
    return gather_out(res.results)


# revision 37
# speedup vs baseline: 1.2816x; 1.0177x over previous
"""BertSelfAttention Trainium2 Bass kernel (v4: fully-overlapped pipeline).

Problem: S=2048, B=4, H=1024, NH=16, DH=64, fp32.
  q/k/v = hidden @ W{q,k,v}.T + b   -> softmax((q k^T)/8 + mask) @ v

Sharding over 8 cores: batch (4) x head-group (2 groups of 8 heads).
Each core gets x=[2048,1024] (its batch), W shards [512,1024] (its 8
heads), mask [2048], and produces outT=[512,2048] (feature-major) which
the host transposes and scatters into the full [S,B,H] output.

The kernel is exp-bound: 256 ScalarE activations of [128,1024] at
~1.34us each are the hard floor. v4 hides everything else behind that
stream:
  - x/W are cast fp32->bf16 by gpsimd DMAs; x and group-0 W transposes
    run as PE 4-block packs (bf16, 1 cyc/row); Wq/Wk groups 1-3 go
    through the XBAR dma_start_transpose (measured ~26GB/s serial on
    HW, fine for late-needed weights, frees the PE)
  - a flat 256-iteration stream emits ACT(t), scores(t+1), PV(t-4):
    the 4-tile PV lag (deep ex buffering) lets the exp stream run ahead
    while V projections are still being produced in block 0
  - two independent generators interleave production into the loop:
    gen-A (x transpose packs + K/Q chains, gating scores) and gen-B
    (Wv packs + V chains, gating only PV), each with its own PSUM bank,
    pulled by need() milestones so kt pulls never drag V work in early
  - PV accumulates [1+64, 512] per head with a leading ones-row (the
    softmax denominator lands in PSUM partition 0 for free); the
    epilogue inverts that row in place (RECIPROCAL_APPROX_FAST on
    [1,512], all partition offsets 0 -- offset-mismatched DVE operands
    return garbage on HW), broadcasts it across partitions with
    gpsimd.partition_broadcast, multiplies on DVE, and DMAs the [d, q]
    tile out feature-major on the sync queue; the host transposes
    during gather (off-device). The recip/bcast/mul/DMA part is
    deferred a few iterations so the PE never stalls at boundaries.
"""

import numpy as np

import concourse.bass as bass
import concourse.mybir as mybir
import concourse.tile as tile
from concourse import bacc
from concourse.bass_utils import run_bass_kernel_spmd
from concourse.masks import make_identity

F32 = mybir.dt.float32
BF16 = mybir.dt.bfloat16
AF = mybir.ActivationFunctionType

S, B, H, NH, DH = 2048, 4, 1024, 16, 64
N_CORES = 8
HPC = 8            # heads per core
DPC = HPC * DH     # 512 output features per core
SC = S // 128      # 16 s-chunks
FC = H // 128      # 8 feature chunks
QG = S // 512      # 4 query groups
KC = S // 128      # 16 key chunks
NG = 4             # head-pair groups per core
LAG = 8            # PV trails ACT by this many tiles
AHEAD = 4          # scores are emitted this many tiles ahead of ACT


def _emit(ctx, tc, nc, x, mask, wq, bq, wk, bk, wv, bv, outT):
    import os
    dbg_aps = getattr(nc, "_dbg_aps", None) if os.environ.get("K_DEBUG") else None

    const_p = ctx.enter_context(tc.tile_pool(name="const", bufs=1))
    xstage_p = ctx.enter_context(tc.tile_pool(name="xstage", bufs=8))
    wstage_p = ctx.enter_context(tc.tile_pool(name="wstage", bufs=6))
    wstgb_p = ctx.enter_context(tc.tile_pool(name="wstgb", bufs=6))
    xt_p = ctx.enter_context(tc.tile_pool(name="xt", bufs=1))
    wvt_p = ctx.enter_context(tc.tile_pool(name="wvt", bufs=1))
    wt_p = ctx.enter_context(tc.tile_pool(name="wt", bufs=8))
    v_p = ctx.enter_context(tc.tile_pool(name="v", bufs=SC))
    qkt_p = ctx.enter_context(tc.tile_pool(name="qkt", bufs=4))
    ex_p = ctx.enter_context(tc.tile_pool(name="ex", bufs=12))
    ctxs_p = ctx.enter_context(tc.tile_pool(name="ctxs", bufs=2))
    rec_p = ctx.enter_context(tc.tile_pool(name="rec", bufs=2))
    bcs_p = ctx.enter_context(tc.tile_pool(name="bcs", bufs=2))
    outt_p = ctx.enter_context(tc.tile_pool(name="outt", bufs=4))

    # psum (8 banks): mm 2x2 (score tiles) + ctx 2x1 (PV accumulators /
    # prologue packs+chains) + qa 1 (gen-A) + qb 1 (gen-B)
    psum_mm = ctx.enter_context(tc.tile_pool(name="psmm", bufs=2, space="PSUM"))
    psum_ctx = ctx.enter_context(tc.tile_pool(name="psctx", bufs=2, space="PSUM"))
    psum_qa = ctx.enter_context(tc.tile_pool(name="psqa", bufs=1, space="PSUM"))
    psum_qb = ctx.enter_context(tc.tile_pool(name="psqb", bufs=1, space="PSUM"))

    # ---- constants ----
    mask_sb = const_p.tile([128, KC], F32)
    nc.sync.dma_start(out=mask_sb, in_=mask.rearrange("(c p) -> p c", p=128))

    ident = const_p.tile([128, 128], F32)
    make_identity(nc, ident)

    ones_col_f = const_p.tile([128, HPC, 1], F32)
    nc.vector.memset(ones_col_f, 1.0)
    # q/k biases as per-partition scalars [128, g] (features on partitions);
    # added during the chain's PSUM->SBUF copy, not on the PE
    bq_sb = const_p.tile([128, NG], F32)
    nc.sync.dma_start(out=bq_sb, in_=bq.rearrange("(g p) -> p g", p=128))
    bk_sb = const_p.tile([128, NG], F32)
    nc.sync.dma_start(out=bk_sb, in_=bk.rearrange("(g p) -> p g", p=128))
    # v bias broadcast across partitions (sequence on partitions); the
    # gpsimd broadcast op itself is emitted from gen-B so it does not
    # block the gpsimd DMA queue during the ramp-up
    bv_row = const_p.tile([1, DPC], F32)
    nc.sync.dma_start(out=bv_row, in_=bv.rearrange("(a f) -> a f", a=1))
    bv_bc = const_p.tile([128, DPC], F32)

    # ---- staging casts (gpsimd DMA, fp32->bf16), priority order ----
    xt = xt_p.tile([128, FC, S], BF16)
    wvt = wvt_p.tile([128, FC, DPC], BF16)
    wqts = [wt_p.tile([128, FC, 128], BF16, tag="wt", name=f"wqt{g}")
            for g in range(NG)]
    wkts = [wt_p.tile([128, FC, 128], BF16, tag="wt", name=f"wkt{g}")
            for g in range(NG)]

    # x and the early-needed weights (Wq0/Wk0/Wv) are staged DIRECTLY in
    # fp32 (read-only DMA, no cast round-trip -- the cast happens on the
    # PSUM->SBUF copy after the PE transpose). This nearly halves the
    # prologue's DMA bytes, which pace the whole ramp-up.
    x_nat = [xstage_p.tile([128, H], F32, tag="xs", name=f"xn{sc}")
             for sc in range(SC)]
    w_nat = {}
    for key, wsrc, gs in (("q", wq, [0]), ("k", wk, [0]),
                          ("v", wv, [0, 1, 2, 3])):
        for g in gs:
            nat = wstage_p.tile([128, H], F32, tag="ws", name=f"wn_{key}{g}")
            w_nat[(key, g)] = nat
    # bf16 staging for the XBAR-transposed late W groups
    for g in range(1, NG):
        for key in ("q", "k"):
            nat = wstgb_p.tile([128, H], BF16, tag="wb", name=f"wb_{key}{g}")
            w_nat[(key, g)] = nat

    def x_dma(sc):
        nc.gpsimd.dma_start(out=x_nat[sc], in_=x[sc * 128:(sc + 1) * 128, :])

    # upfront DMAs: x0-3, wq0, wk0, x4-7, wv0-3 (fp32, priority order --
    # exactly the order the PE prologue consumes them)
    for sc in range(4):
        x_dma(sc)
    nc.gpsimd.dma_start(out=w_nat[("q", 0)], in_=wq[0:128, :])
    nc.gpsimd.dma_start(out=w_nat[("k", 0)], in_=wk[0:128, :])
    for sc in range(4, 8):
        x_dma(sc)
    for dc in range(4):
        nc.gpsimd.dma_start(out=w_nat[("v", dc)],
                            in_=wv[dc * 128:(dc + 1) * 128, :])
    # x8-15 DMAs are emitted as buffers free (after tp_x(sc-8));
    # Wq/Wk g1-3 casts + XBAR transposes are emitted from inside the
    # attention loop so they never compete with the ramp-up DMA.

    def emit_wg_late(g):
        for key, wsrc in (("q", wq), ("k", wk)):
            nat = w_nat[(key, g)]
            nc.gpsimd.dma_start(out=nat, in_=wsrc[g * 128:(g + 1) * 128, :])
        for key, dst in (("q", wqts[g]), ("k", wkts[g])):
            nat = w_nat[(key, g)]
            for fc in range(FC):
                nc.sync.dma_start_transpose(dst[:, fc, :],
                                            nat[:, fc * 128:(fc + 1) * 128])

    # ---- PE transpose packs (fp32 in, bf16 out via the copy) ----
    def tp_pack(dst_view, src_nat, fc0, pool, tag):
        pt = pool.tile([128, 4, 128], F32, tag=tag, name="pt")
        for j in range(4):
            fc = fc0 + j
            nc.tensor.transpose(pt[:, j, :],
                                src_nat[:, fc * 128:(fc + 1) * 128], ident)
            yield
        nc.vector.tensor_copy(dst_view, pt)
        yield

    def tp_x(sc, pool, tag):
        for fc0 in (0, 4):
            yield from tp_pack(xt[:, fc0:fc0 + 4, sc * 128:(sc + 1) * 128],
                               x_nat[sc], fc0, pool, tag)
        if sc + 8 < SC:
            x_dma(sc + 8)

    def tp_wqk(key, g, pool, tag):
        dst = wqts[g] if key == "q" else wkts[g]
        for fc0 in (0, 4):
            yield from tp_pack(dst[:, fc0:fc0 + 4, :], w_nat[(key, g)],
                               fc0, pool, tag)

    def tp_wv(dc, pool, tag):
        for fc0 in (0, 4):
            yield from tp_pack(wvt[:, fc0:fc0 + 4, dc * 128:(dc + 1) * 128],
                               w_nat[("v", dc)], fc0, pool, tag)

    # ---- projection chains ----
    # v_sb layout: [:, h, 0] = ones (denominator row), [:, h, 1:65] = V
    v_sb = [v_p.tile([128, HPC, DH + 1], BF16, tag="v", name=f"v{sc}")
            for sc in range(SC)]
    qts = {}
    kts = {}

    def get_qkt(kind, g):
        d = qts if kind == "qt" else kts
        if g not in d:
            d[g] = qkt_p.tile([128, S], BF16, tag="qkt", name=f"{kind}{g}")
        return d[g]

    def v_chain(sc, pool, tag):
        vp = pool.tile([128, DPC], F32, tag=tag, name=f"vp{sc}")
        for fc in range(FC):
            nc.tensor.matmul(vp, xt[:, fc, sc * 128:(sc + 1) * 128],
                             wvt[:, fc, :], start=(fc == 0),
                             stop=(fc == FC - 1))
            yield
        nc.gpsimd.tensor_copy(v_sb[sc][:, :, 0:1], ones_col_f)
        nc.vector.tensor_add(v_sb[sc][:, :, 1:DH + 1],
                             vp.rearrange("p (h d) -> p h d", d=DH),
                             bv_bc.rearrange("p (h d) -> p h d", d=DH))
        yield

    def qk_chain(kind, g, sg, pool, tag):
        bias_sb = bq_sb if kind == "qt" else bk_sb
        wt_src = wqts[g] if kind == "qt" else wkts[g]
        qk_dst = get_qkt(kind, g)
        ssl = slice(sg * 512, (sg + 1) * 512)
        qp = pool.tile([128, 512], F32, tag=tag, name=f"{kind}{g}s{sg}p")
        for fc in range(FC):
            nc.tensor.matmul(qp, wt_src[:, fc, :], xt[:, fc, ssl],
                             start=(fc == 0), stop=(fc == FC - 1))
            yield
        nc.vector.tensor_scalar_add(qk_dst[:, ssl], qp, bias_sb[:, g:g + 1])
        yield

    done = set()

    def run_now(gen_):
        for _ in gen_:
            pass

    # ---- prologue: minimum to start the exp stream (PE order matches
    # the DMA arrival order: x0-3 first, then wq0/wk0) ----
    for sc in range(4):
        run_now(tp_x(sc, psum_ctx, "ctx"))
    run_now(tp_wqk("q", 0, psum_ctx, "ctx"))
    run_now(tp_wqk("k", 0, psum_ctx, "ctx"))
    run_now(qk_chain("kt", 0, 0, psum_ctx, "ctx"))
    run_now(qk_chain("qt", 0, 0, psum_ctx, "ctx"))
    done.update({"kt0s0", "qt0s0"})

    # ---- generators: A gates scores (kt/qt), B gates PV (v) ----
    plan_a = [("x", 4), ("x", 5), ("x", 6), ("x", 7), ("kt", 0, 1),
              ("x", 8), ("x", 9), ("x", 10), ("x", 11), ("kt", 0, 2),
              ("x", 12), ("x", 13), ("x", 14), ("x", 15), ("kt", 0, 3),
              ("qt", 0, 1), ("qt", 0, 2), ("qt", 0, 3)]
    for g in range(1, NG):
        plan_a += [("wgate", g), ("kt", g, 0), ("qt", g, 0), ("kt", g, 1),
                   ("kt", g, 2), ("kt", g, 3), ("qt", g, 1), ("qt", g, 2),
                   ("qt", g, 3)]
    wg_emitted = set()
    plan_b = ([("bvbc",)] + [("wv", dc) for dc in range(4)] +
              [("v", sc) for sc in range(SC)])

    def run_plan(plan, pool, tag):
        for item in plan:
            if item[0] == "wgate":
                # stall (empty yields) until the flat loop has emitted the
                # casts + XBAR transposes for this W group
                while item[1] not in wg_emitted:
                    yield
            elif item[0] == "bvbc":
                nc.gpsimd.partition_broadcast(bv_bc, bv_row)
                yield
            elif item[0] == "x":
                yield from tp_x(item[1], pool, tag)
                done.add(f"x{item[1]}")
            elif item[0] == "wv":
                yield from tp_wv(item[1], pool, tag)
            elif item[0] == "v":
                sc = item[1]
                if sc >= 4:
                    # xt[sc] comes from gen-A: force its pack to be
                    # emitted first (cross-generator RAW dependency)
                    need(0, f"x{sc}")
                yield from v_chain(sc, pool, tag)
                done.add(f"v{sc}")
            else:
                kind, g, sg = item
                yield from qk_chain(kind, g, sg, pool, tag)
                done.add(f"{kind}{g}s{sg}")

    gens = [run_plan(plan_a, psum_qa, "qa"), run_plan(plan_b, psum_qb, "qb")]

    def drive(n):
        # round-robin both generators
        for _ in range(n):
            alive = [g for g in gens if g is not None]
            if not alive:
                return
            for idx in range(2):
                if gens[idx] is None:
                    continue
                try:
                    next(gens[idx])
                except StopIteration:
                    gens[idx] = None

    def need(idx, *products):
        while gens[idx] is not None and not all(p in done for p in products):
            try:
                next(gens[idx])
            except StopIteration:
                gens[idx] = None

    # ---- attention: flat pipelined stream ----
    blocks = [(g2, qg) for g2 in range(NG) for qg in range(QG)]
    T = len(blocks) * KC
    pend_st = {}
    cur_cp = {}

    def emit_scores(t):
        bi, kc = divmod(t, KC)
        g2, qg = blocks[bi]
        qt, kt = get_qkt("qt", g2), get_qkt("kt", g2)
        ksl = slice(kc * 128, (kc + 1) * 128)
        qsl = slice(qg * 512, (qg + 1) * 512)
        st = psum_mm.tile([128, 2, 512], F32, tag="mm")
        nc.tensor.matmul(st[:, 0, :], kt[0:64, ksl], qt[0:64, qsl],
                         start=True, stop=True)
        nc.tensor.matmul(st[:, 1, :], kt[64:128, ksl], qt[64:128, qsl],
                         start=True, stop=True)
        pend_st[t] = st

    pend_ex = {}
    post = []   # deferred epilogue closures (recip/bcast/mul/dma)

    if dbg_aps:
        dbg2_p = ctx.enter_context(tc.tile_pool(name="dbgt", bufs=1))
        dbg_ex = dbg2_p.tile([128, 2, 512], BF16, tag="dx", name="dbg_ex")
        dbg_ctxs = dbg2_p.tile([DH + 1, 512], F32, tag="dc", name="dbg_ctxs")
        dbg_rec = dbg2_p.tile([1, 512], F32, tag="dr", name="dbg_rec")
        dbg_bc = dbg2_p.tile([DH + 1, 512], F32, tag="db", name="dbg_bc")

    def epilogue_a(bi, cp0, cp1):
        g2, qg = blocks[bi]
        qsl = slice(qg * 512, (qg + 1) * 512)
        for h_loc, cp in ((0, cp0), (1, cp1)):
            h = 2 * g2 + h_loc
            ctxs = ctxs_p.tile([DH + 1, 512], F32, tag="ctxs")
            nc.vector.tensor_copy(ctxs, cp)
            first = (bi == 0 and h_loc == 0)
            if dbg_aps and first:
                nc.vector.tensor_copy(dbg_ctxs, ctxs)

            def fin(h=h, ctxs=ctxs, qsl=qsl, first=first):
                rec = rec_p.tile([1, 512], F32, tag="rec")
                nc.vector.reciprocal_approx_fast(rec, ctxs[0:1, :])
                bc = bcs_p.tile([DH + 1, 512], F32, tag="bc")
                nc.gpsimd.partition_broadcast(bc, rec)
                ot = outt_p.tile([DH + 1, 512], F32, tag="outt")
                nc.vector.tensor_mul(ot, ctxs, bc)
                nc.sync.dma_start(out=outT[h * DH:(h + 1) * DH, qsl],
                                  in_=ot[1:DH + 1, :])
                if dbg_aps and first:
                    nc.vector.tensor_copy(dbg_rec, rec)
                    nc.vector.tensor_copy(dbg_bc, bc)

            post.append(fin)

    for k in range(AHEAD):
        emit_scores(k)
    drive(12)
    for t in range(T + LAG):
        if t == 28:
            emit_wg_late(1)
            wg_emitted.add(1)
        elif t == 64:
            emit_wg_late(2)
            wg_emitted.add(2)
        elif t == 100:
            emit_wg_late(3)
            wg_emitted.add(3)
        if t < T:
            bi, kc = divmod(t, KC)
            st = pend_st.pop(t)
            ex = ex_p.tile([128, 2, 512], BF16, tag="ex")
            nc.scalar.activation(ex.rearrange("p a b -> p (a b)"),
                                 st.rearrange("p a b -> p (a b)"),
                                 AF.Exp, bias=mask_sb[:, kc:kc + 1],
                                 scale=1.0 / np.sqrt(DH))
            pend_ex[t] = ex
            if dbg_aps and t == 0:
                nc.gpsimd.tensor_copy(dbg_ex, ex)
            if t + AHEAD < T:
                nbi, nkc = divmod(t + AHEAD, KC)
                ng2, nqg = blocks[nbi]
                need(0, f"kt{ng2}s{nkc // 4}", f"qt{ng2}s{nqg}")
                emit_scores(t + AHEAD)
        pt_ = t - LAG
        if pt_ >= 0:
            pbi, pkc = divmod(pt_, KC)
            pg2, _ = blocks[pbi]
            if pkc == 0:
                cpa = psum_ctx.tile([DH + 1, 512], F32, tag="ctx", name="cpa")
                cpb = psum_ctx.tile([DH + 1, 512], F32, tag="ctx", name="cpb")
                cur_cp[pbi] = (cpa, cpb)
            cp0, cp1 = cur_cp[pbi]
            if pbi == 0:
                need(1, f"v{pkc}")
            ex = pend_ex.pop(pt_)
            nc.tensor.matmul(cp0, v_sb[pkc][:, 2 * pg2, :], ex[:, 0, :],
                             start=(pkc == 0), stop=(pkc == KC - 1))
            nc.tensor.matmul(cp1, v_sb[pkc][:, 2 * pg2 + 1, :], ex[:, 1, :],
                             start=(pkc == 0), stop=(pkc == KC - 1))
            if pkc == KC - 1:
                epilogue_a(pbi, cp0, cp1)
                del cur_cp[pbi]
        if (t % KC) in (9, 13) and post:
            post.pop(0)()
        drive(2)

    while post:
        post.pop(0)()
    for idx in range(2):
        while gens[idx] is not None:
            try:
                next(gens[idx])
            except StopIteration:
                gens[idx] = None

    if dbg_aps:
        nc.sync.dma_start(out=dbg_aps["xt"], in_=xt)
        nc.sync.dma_start(out=dbg_aps["qt0"], in_=qts[0])
        nc.sync.dma_start(out=dbg_aps["kt0"], in_=kts[0])
        nc.sync.dma_start(out=dbg_aps["v0"], in_=v_sb[0])
        nc.sync.dma_start(out=dbg_aps["ex00"], in_=dbg_ex)
        nc.sync.dma_start(out=dbg_aps["ctxs0"], in_=dbg_ctxs)
        nc.sync.dma_start(out=dbg_aps["rec0"], in_=dbg_rec)
        nc.sync.dma_start(out=dbg_aps["bc0"], in_=dbg_bc)


def build_program():
    nc = bacc.Bacc("TRN2", target_bir_lowering=False, debug=False)
    x = nc.dram_tensor("x", [S, H], F32, kind="ExternalInput").ap()
    mask = nc.dram_tensor("mask", [S], F32, kind="ExternalInput").ap()
    wq = nc.dram_tensor("wq", [DPC, H], F32, kind="ExternalInput").ap()
    bq = nc.dram_tensor("bq", [DPC], F32, kind="ExternalInput").ap()
    wk = nc.dram_tensor("wk", [DPC, H], F32, kind="ExternalInput").ap()
    bk = nc.dram_tensor("bk", [DPC], F32, kind="ExternalInput").ap()
    wv = nc.dram_tensor("wv", [DPC, H], F32, kind="ExternalInput").ap()
    bv = nc.dram_tensor("bv", [DPC], F32, kind="ExternalInput").ap()
    outT = nc.dram_tensor("outT", [DPC, S], F32, kind="ExternalOutput").ap()

    import os
    if os.environ.get("K_DEBUG"):
        nc._dbg_aps = {
            "xt": nc.dram_tensor("xt_dbg", [128, FC, S], BF16,
                                 kind="ExternalOutput").ap(),
            "qt0": nc.dram_tensor("qt0_dbg", [128, S], BF16,
                                  kind="ExternalOutput").ap(),
            "kt0": nc.dram_tensor("kt0_dbg", [128, S], BF16,
                                  kind="ExternalOutput").ap(),
            "v0": nc.dram_tensor("v0_dbg", [128, HPC, DH + 1], BF16,
                                 kind="ExternalOutput").ap(),
            "ex00": nc.dram_tensor("ex00_dbg", [128, 2, 512], BF16,
                                   kind="ExternalOutput").ap(),
            "ctxs0": nc.dram_tensor("ctxs0_dbg", [DH + 1, 512], F32,
                                    kind="ExternalOutput").ap(),
            "rec0": nc.dram_tensor("rec0_dbg", [1, 512], F32,
                                   kind="ExternalOutput").ap(),
            "bc0": nc.dram_tensor("bc0_dbg", [DH + 1, 512], F32,
                                  kind="ExternalOutput").ap(),
        }

    from contextlib import ExitStack
    with tile.TileContext(nc) as tc:
        with ExitStack() as ctx:
            _emit(ctx, tc, nc, x, mask, wq, bq, wk, bk, wv, bv, outT)
    nc.compile()
    return nc


_NC_CACHE = None


def make_in_maps(hidden_states, attention_mask, Wq, bq, Wk, bk, Wv, bv):
    hs = np.asarray(hidden_states, dtype=np.float32)
    am = np.asarray(attention_mask, dtype=np.float32)
    ws = {k: np.asarray(v, dtype=np.float32)
          for k, v in (("wq", Wq), ("bq", bq), ("wk", Wk),
                       ("bk", bk), ("wv", Wv), ("bv", bv))}
    in_maps = []
    for c in range(N_CORES):
        b, g = divmod(c, 2)
        sl = slice(g * DPC, (g + 1) * DPC)
        in_maps.append({
            "x": np.ascontiguousarray(hs[:, b, :]),
            "mask": np.ascontiguousarray(am[b, 0, 0, :]),
            "wq": np.ascontiguousarray(ws["wq"][sl]),
            "bq": np.ascontiguousarray(ws["bq"][sl]),
            "wk": np.ascontiguousarray(ws["wk"][sl]),
            "bk": np.ascontiguousarray(ws["bk"][sl]),
            "wv": np.ascontiguousarray(ws["wv"][sl]),
            "bv": np.ascontiguousarray(ws["bv"][sl]),
        })
    return in_maps


def gather_out(results):
    out = np.empty((S, B, H), np.float32)
    for c in range(N_CORES):
        b, g = divmod(c, 2)
        out[:, b, g * DPC:(g + 1) * DPC] = results[c]["outT"].T
    return out


def kernel(hidden_states, attention_mask, Wq, bq, Wk, bk, Wv, bv):
    global _NC_CACHE
    if _NC_CACHE is None:
        _NC_CACHE = build_program()
    in_maps = make_in_maps(hidden_states, attention_mask,
                           Wq, bq, Wk, bk, Wv, bv)
    res = run_bass_kernel_spmd(_NC_CACHE, in_maps, list(range(N_CORES)))
    return gather_out(res.results)


# revision 40
# speedup vs baseline: 1.2981x; 1.0129x over previous
"""BertSelfAttention Trainium2 Bass kernel (v4: fully-overlapped pipeline).

Problem: S=2048, B=4, H=1024, NH=16, DH=64, fp32.
  q/k/v = hidden @ W{q,k,v}.T + b   -> softmax((q k^T)/8 + mask) @ v

Sharding over 8 cores: batch (4) x head-group (2 groups of 8 heads).
Each core gets x=[2048,1024] (its batch), W shards [512,1024] (its 8
heads), mask [2048], and produces outT=[512,2048] (feature-major) which
the host transposes and scatters into the full [S,B,H] output.

The kernel is exp-bound: 256 ScalarE activations of [128,1024] at
~1.34us each are the hard floor. v4 hides everything else behind that
stream:
  - x/W are cast fp32->bf16 by gpsimd DMAs; x and group-0 W transposes
    run as PE 4-block packs (bf16, 1 cyc/row); Wq/Wk groups 1-3 go
    through the XBAR dma_start_transpose (measured ~26GB/s serial on
    HW, fine for late-needed weights, frees the PE)
  - a flat 256-iteration stream emits ACT(t), scores(t+1), PV(t-4):
    the 4-tile PV lag (deep ex buffering) lets the exp stream run ahead
    while V projections are still being produced in block 0
  - two independent generators interleave production into the loop:
    gen-A (x transpose packs + K/Q chains, gating scores) and gen-B
    (Wv packs + V chains, gating only PV), each with its own PSUM bank,
    pulled by need() milestones so kt pulls never drag V work in early
  - PV accumulates [1+64, 512] per head with a leading ones-row (the
    softmax denominator lands in PSUM partition 0 for free); the
    epilogue inverts that row in place (RECIPROCAL_APPROX_FAST on
    [1,512], all partition offsets 0 -- offset-mismatched DVE operands
    return garbage on HW), broadcasts it across partitions with
    gpsimd.partition_broadcast, multiplies on DVE, and DMAs the [d, q]
    tile out feature-major on the sync queue; the host transposes
    during gather (off-device). The recip/bcast/mul/DMA part is
    deferred a few iterations so the PE never stalls at boundaries.
"""

import numpy as np

import concourse.bass as bass
import concourse.mybir as mybir
import concourse.tile as tile
from concourse import bacc
from concourse.bass_utils import run_bass_kernel_spmd
from concourse.masks import make_identity

F32 = mybir.dt.float32
BF16 = mybir.dt.bfloat16
AF = mybir.ActivationFunctionType

S, B, H, NH, DH = 2048, 4, 1024, 16, 64
N_CORES = 8
HPC = 8            # heads per core
DPC = HPC * DH     # 512 output features per core
SC = S // 128      # 16 s-chunks
FC = H // 128      # 8 feature chunks
QG = S // 512      # 4 query groups
KC = S // 128      # 16 key chunks
NG = 4             # head-pair groups per core
LAG = 8            # PV trails ACT by this many tiles
AHEAD = 4          # scores are emitted this many tiles ahead of ACT


def _emit(ctx, tc, nc, x, mask, wq, bq, wk, bk, wv, bv, outT):
    import os
    dbg_aps = getattr(nc, "_dbg_aps", None) if os.environ.get("K_DEBUG") else None

    const_p = ctx.enter_context(tc.tile_pool(name="const", bufs=1))
    xstage_p = ctx.enter_context(tc.tile_pool(name="xstage", bufs=8))
    wstage_p = ctx.enter_context(tc.tile_pool(name="wstage", bufs=6))
    wstgb_p = ctx.enter_context(tc.tile_pool(name="wstgb", bufs=6))
    xt_p = ctx.enter_context(tc.tile_pool(name="xt", bufs=1))
    wvt_p = ctx.enter_context(tc.tile_pool(name="wvt", bufs=1))
    wt_p = ctx.enter_context(tc.tile_pool(name="wt", bufs=8))
    v_p = ctx.enter_context(tc.tile_pool(name="v", bufs=SC))
    qkt_p = ctx.enter_context(tc.tile_pool(name="qkt", bufs=4))
    ex_p = ctx.enter_context(tc.tile_pool(name="ex", bufs=12))
    ctxs_p = ctx.enter_context(tc.tile_pool(name="ctxs", bufs=2))
    rec_p = ctx.enter_context(tc.tile_pool(name="rec", bufs=2))
    bcs_p = ctx.enter_context(tc.tile_pool(name="bcs", bufs=2))
    outt_p = ctx.enter_context(tc.tile_pool(name="outt", bufs=4))

    # psum (8 banks): mm 2x2 (score tiles) + ctx 2x1 (PV accumulators /
    # prologue packs+chains) + qa 1 (gen-A) + qb 1 (gen-B)
    psum_mm = ctx.enter_context(tc.tile_pool(name="psmm", bufs=2, space="PSUM"))
    psum_ctx = ctx.enter_context(tc.tile_pool(name="psctx", bufs=2, space="PSUM"))
    psum_qa = ctx.enter_context(tc.tile_pool(name="psqa", bufs=1, space="PSUM"))
    psum_qb = ctx.enter_context(tc.tile_pool(name="psqb", bufs=1, space="PSUM"))

    # ---- constants ----
    mask_sb = const_p.tile([128, KC], F32)
    nc.sync.dma_start(out=mask_sb, in_=mask.rearrange("(c p) -> p c", p=128))

    ident = const_p.tile([128, 128], F32)
    make_identity(nc, ident)
    ident_bf = const_p.tile([128, 128], BF16)
    nc.vector.tensor_copy(ident_bf, ident)

    ones_col_f = const_p.tile([128, HPC, 1], F32)
    nc.vector.memset(ones_col_f, 1.0)
    # q/k biases as per-partition scalars [128, g] (features on partitions);
    # added during the chain's PSUM->SBUF copy, not on the PE
    bq_sb = const_p.tile([128, NG], F32)
    nc.sync.dma_start(out=bq_sb, in_=bq.rearrange("(g p) -> p g", p=128))
    bk_sb = const_p.tile([128, NG], F32)
    nc.sync.dma_start(out=bk_sb, in_=bk.rearrange("(g p) -> p g", p=128))
    # v bias broadcast across partitions (sequence on partitions); the
    # gpsimd broadcast op itself is emitted from gen-B so it does not
    # block the gpsimd DMA queue during the ramp-up
    bv_row = const_p.tile([1, DPC], F32)
    nc.sync.dma_start(out=bv_row, in_=bv.rearrange("(a f) -> a f", a=1))
    bv_bc = const_p.tile([128, DPC], F32)

    # ---- staging casts (gpsimd DMA, fp32->bf16), priority order ----
    xt = xt_p.tile([128, FC, S], BF16)
    wvt = wvt_p.tile([128, FC, DPC], BF16)
    wqts = [wt_p.tile([128, FC, 128], BF16, tag="wt", name=f"wqt{g}")
            for g in range(NG)]
    wkts = [wt_p.tile([128, FC, 128], BF16, tag="wt", name=f"wkt{g}")
            for g in range(NG)]

    # x and the early-needed weights (Wq0/Wk0/Wv) are staged DIRECTLY in
    # fp32 (read-only DMA, no cast round-trip -- the cast happens on the
    # PSUM->SBUF copy after the PE transpose). This nearly halves the
    # prologue's DMA bytes, which pace the whole ramp-up.
    # chunks 0-7 staged fp32 (prologue latency path); 8-15 staged bf16
    # via cast DMA (off the critical path, halves the PE transpose time)
    x_nat = [xstage_p.tile([128, H], F32 if sc < 8 else BF16,
                           tag="xs", name=f"xn{sc}")
             for sc in range(SC)]
    w_nat = {}
    for key, wsrc, gs in (("q", wq, [0]), ("k", wk, [0]),
                          ("v", wv, [0, 1, 2, 3])):
        for g in gs:
            nat = wstage_p.tile([128, H], F32, tag="ws", name=f"wn_{key}{g}")
            w_nat[(key, g)] = nat
    # bf16 staging for the XBAR-transposed late W groups
    for g in range(1, NG):
        for key in ("q", "k"):
            nat = wstgb_p.tile([128, H], BF16, tag="wb", name=f"wb_{key}{g}")
            w_nat[(key, g)] = nat

    def x_dma(sc):
        nc.gpsimd.dma_start(out=x_nat[sc], in_=x[sc * 128:(sc + 1) * 128, :])

    # upfront DMAs: x0-3, wq0, wk0, x4-7, wv0-3 (fp32, priority order --
    # exactly the order the PE prologue consumes them)
    for sc in range(4):
        x_dma(sc)
    nc.gpsimd.dma_start(out=w_nat[("q", 0)], in_=wq[0:128, :])
    nc.gpsimd.dma_start(out=w_nat[("k", 0)], in_=wk[0:128, :])
    for sc in range(4, 8):
        x_dma(sc)
    for dc in range(4):
        nc.gpsimd.dma_start(out=w_nat[("v", dc)],
                            in_=wv[dc * 128:(dc + 1) * 128, :])
    # x8-15 DMAs are emitted as buffers free (after tp_x(sc-8));
    # Wq/Wk g1-3 casts + XBAR transposes are emitted from inside the
    # attention loop so they never compete with the ramp-up DMA.

    def emit_wg_late(g):
        for key, wsrc in (("q", wq), ("k", wk)):
            nat = w_nat[(key, g)]
            nc.gpsimd.dma_start(out=nat, in_=wsrc[g * 128:(g + 1) * 128, :])
        for key, dst in (("q", wqts[g]), ("k", wkts[g])):
            nat = w_nat[(key, g)]
            for fc in range(FC):
                nc.sync.dma_start_transpose(dst[:, fc, :],
                                            nat[:, fc * 128:(fc + 1) * 128])

    # ---- PE transpose packs (fp32 or bf16 in, bf16 out via the copy) ----
    def tp_pack(dst_view, src_nat, fc0, pool, tag):
        bf = src_nat.dtype == BF16
        pt = pool.tile([128, 4, 128], BF16 if bf else F32, tag=tag, name="pt")
        for j in range(4):
            fc = fc0 + j
            nc.tensor.transpose(pt[:, j, :],
                                src_nat[:, fc * 128:(fc + 1) * 128],
                                ident_bf if bf else ident)
            yield
        nc.vector.tensor_copy(dst_view, pt)
        yield

    def tp_x(sc, pool, tag):
        for fc0 in (0, 4):
            yield from tp_pack(xt[:, fc0:fc0 + 4, sc * 128:(sc + 1) * 128],
                               x_nat[sc], fc0, pool, tag)
        if sc + 8 < SC:
            x_dma(sc + 8)

    def tp_wqk(key, g, pool, tag):
        dst = wqts[g] if key == "q" else wkts[g]
        for fc0 in (0, 4):
            yield from tp_pack(dst[:, fc0:fc0 + 4, :], w_nat[(key, g)],
                               fc0, pool, tag)

    def tp_wv(dc, pool, tag):
        for fc0 in (0, 4):
            yield from tp_pack(wvt[:, fc0:fc0 + 4, dc * 128:(dc + 1) * 128],
                               w_nat[("v", dc)], fc0, pool, tag)

    # ---- projection chains ----
    # v_sb layout: [:, h, 0] = ones (denominator row), [:, h, 1:65] = V
    v_sb = [v_p.tile([128, HPC, DH + 1], BF16, tag="v", name=f"v{sc}")
            for sc in range(SC)]
    qts = {}
    kts = {}

    def get_qkt(kind, g):
        d = qts if kind == "qt" else kts
        if g not in d:
            d[g] = qkt_p.tile([128, S], BF16, tag="qkt", name=f"{kind}{g}")
        return d[g]

    def v_chain(sc, pool, tag):
        vp = pool.tile([128, DPC], F32, tag=tag, name=f"vp{sc}")
        for fc in range(FC):
            nc.tensor.matmul(vp, xt[:, fc, sc * 128:(sc + 1) * 128],
                             wvt[:, fc, :], start=(fc == 0),
                             stop=(fc == FC - 1))
            yield
        nc.gpsimd.tensor_copy(v_sb[sc][:, :, 0:1], ones_col_f)
        nc.vector.tensor_add(v_sb[sc][:, :, 1:DH + 1],
                             vp.rearrange("p (h d) -> p h d", d=DH),
                             bv_bc.rearrange("p (h d) -> p h d", d=DH))
        yield

    def qk_chain(kind, g, sg, pool, tag):
        bias_sb = bq_sb if kind == "qt" else bk_sb
        wt_src = wqts[g] if kind == "qt" else wkts[g]
        qk_dst = get_qkt(kind, g)
        ssl = slice(sg * 512, (sg + 1) * 512)
        qp = pool.tile([128, 512], F32, tag=tag, name=f"{kind}{g}s{sg}p")
        for fc in range(FC):
            nc.tensor.matmul(qp, wt_src[:, fc, :], xt[:, fc, ssl],
                             start=(fc == 0), stop=(fc == FC - 1))
            yield
        nc.vector.tensor_scalar_add(qk_dst[:, ssl], qp, bias_sb[:, g:g + 1])
        yield

    done = set()

    def run_now(gen_):
        for _ in gen_:
            pass

    # ---- prologue: minimum to start the exp stream (PE order matches
    # the DMA arrival order: x0-3 first, then wq0/wk0) ----
    for sc in range(4):
        run_now(tp_x(sc, psum_ctx, "ctx"))
    run_now(tp_wqk("q", 0, psum_ctx, "ctx"))
    run_now(tp_wqk("k", 0, psum_ctx, "ctx"))
    run_now(qk_chain("kt", 0, 0, psum_ctx, "ctx"))
    run_now(qk_chain("qt", 0, 0, psum_ctx, "ctx"))
    done.update({"kt0s0", "qt0s0"})

    # ---- generators: A gates scores (kt/qt), B gates PV (v) ----
    plan_a = [("x", 4), ("x", 5), ("x", 6), ("x", 7), ("kt", 0, 1),
              ("x", 8), ("x", 9), ("x", 10), ("x", 11), ("kt", 0, 2),
              ("x", 12), ("x", 13), ("x", 14), ("x", 15), ("kt", 0, 3),
              ("qt", 0, 1), ("qt", 0, 2), ("qt", 0, 3)]
    for g in range(1, NG):
        plan_a += [("wgate", g), ("kt", g, 0), ("qt", g, 0), ("kt", g, 1),
                   ("kt", g, 2), ("kt", g, 3), ("qt", g, 1), ("qt", g, 2),
                   ("qt", g, 3)]
    wg_emitted = set()
    plan_b = ([("bvbc",)] + [("wv", dc) for dc in range(4)] +
              [("v", sc) for sc in range(SC)])

    def run_plan(plan, pool, tag):
        for item in plan:
            if item[0] == "wgate":
                # stall (empty yields) until the flat loop has emitted the
                # casts + XBAR transposes for this W group
                while item[1] not in wg_emitted:
                    yield
            elif item[0] == "bvbc":
                nc.gpsimd.partition_broadcast(bv_bc, bv_row)
                yield
            elif item[0] == "x":
                yield from tp_x(item[1], pool, tag)
                done.add(f"x{item[1]}")
            elif item[0] == "wv":
                yield from tp_wv(item[1], pool, tag)
            elif item[0] == "v":
                sc = item[1]
                if sc >= 4:
                    # xt[sc] comes from gen-A: force its pack to be
                    # emitted first (cross-generator RAW dependency)
                    need(0, f"x{sc}")
                yield from v_chain(sc, pool, tag)
                done.add(f"v{sc}")
            else:
                kind, g, sg = item
                yield from qk_chain(kind, g, sg, pool, tag)
                done.add(f"{kind}{g}s{sg}")

    gens = [run_plan(plan_a, psum_qa, "qa"), run_plan(plan_b, psum_qb, "qb")]

    def drive(n):
        # round-robin both generators
        for _ in range(n):
            alive = [g for g in gens if g is not None]
            if not alive:
                return
            for idx in range(2):
                if gens[idx] is None:
                    continue
                try:
                    next(gens[idx])
                except StopIteration:
                    gens[idx] = None

    def need(idx, *products):
        while gens[idx] is not None and not all(p in done for p in products):
            try:
                next(gens[idx])
            except StopIteration:
                gens[idx] = None

    # ---- attention: flat pipelined stream ----
    blocks = [(g2, qg) for g2 in range(NG) for qg in range(QG)]
    T = len(blocks) * KC
    pend_st = {}
    cur_cp = {}

    def emit_scores(t):
        bi, kc = divmod(t, KC)
        g2, qg = blocks[bi]
        qt, kt = get_qkt("qt", g2), get_qkt("kt", g2)
        ksl = slice(kc * 128, (kc + 1) * 128)
        qsl = slice(qg * 512, (qg + 1) * 512)
        st = psum_mm.tile([128, 2, 512], F32, tag="mm")
        nc.tensor.matmul(st[:, 0, :], kt[0:64, ksl], qt[0:64, qsl],
                         start=True, stop=True)
        nc.tensor.matmul(st[:, 1, :], kt[64:128, ksl], qt[64:128, qsl],
                         start=True, stop=True)
        pend_st[t] = st

    pend_ex = {}
    post = []   # deferred epilogue closures (recip/bcast/mul/dma)

    if dbg_aps:
        dbg2_p = ctx.enter_context(tc.tile_pool(name="dbgt", bufs=1))
        dbg_ex = dbg2_p.tile([128, 2, 512], BF16, tag="dx", name="dbg_ex")
        dbg_ctxs = dbg2_p.tile([DH + 1, 512], F32, tag="dc", name="dbg_ctxs")
        dbg_rec = dbg2_p.tile([1, 512], F32, tag="dr", name="dbg_rec")
        dbg_bc = dbg2_p.tile([DH + 1, 512], F32, tag="db", name="dbg_bc")

    def epilogue_a(bi, cp0, cp1):
        g2, qg = blocks[bi]
        qsl = slice(qg * 512, (qg + 1) * 512)
        for h_loc, cp in ((0, cp0), (1, cp1)):
            h = 2 * g2 + h_loc
            ctxs = ctxs_p.tile([DH + 1, 512], F32, tag="ctxs")
            nc.vector.tensor_copy(ctxs, cp)
            first = (bi == 0 and h_loc == 0)
            if dbg_aps and first:
                nc.vector.tensor_copy(dbg_ctxs, ctxs)

            def fin(h=h, ctxs=ctxs, qsl=qsl, first=first):
                rec = rec_p.tile([1, 512], F32, tag="rec")
                nc.vector.reciprocal_approx_fast(rec, ctxs[0:1, :])
                bc = bcs_p.tile([DH + 1, 512], F32, tag="bc")
                nc.gpsimd.partition_broadcast(bc, rec)
                ot = outt_p.tile([DH + 1, 512], F32, tag="outt")
                nc.vector.tensor_mul(ot, ctxs, bc)
                nc.sync.dma_start(out=outT[h * DH:(h + 1) * DH, qsl],
                                  in_=ot[1:DH + 1, :])
                if dbg_aps and first:
                    nc.vector.tensor_copy(dbg_rec, rec)
                    nc.vector.tensor_copy(dbg_bc, bc)

            post.append(fin)

    for k in range(AHEAD):
        emit_scores(k)
    drive(12)
    for t in range(T + LAG):
        if t == 28:
            emit_wg_late(1)
            wg_emitted.add(1)
        elif t == 64:
            emit_wg_late(2)
            wg_emitted.add(2)
        elif t == 100:
            emit_wg_late(3)
            wg_emitted.add(3)
        if t < T:
            bi, kc = divmod(t, KC)
            st = pend_st.pop(t)
            ex = ex_p.tile([128, 2, 512], BF16, tag="ex")
            nc.scalar.activation(ex.rearrange("p a b -> p (a b)"),
                                 st.rearrange("p a b -> p (a b)"),
                                 AF.Exp, bias=mask_sb[:, kc:kc + 1],
                                 scale=1.0 / np.sqrt(DH))
            pend_ex[t] = ex
            if dbg_aps and t == 0:
                nc.gpsimd.tensor_copy(dbg_ex, ex)
            if t + AHEAD < T:
                nbi, nkc = divmod(t + AHEAD, KC)
                ng2, nqg = blocks[nbi]
                need(0, f"kt{ng2}s{nkc // 4}", f"qt{ng2}s{nqg}")
                emit_scores(t + AHEAD)
        pt_ = t - LAG
        if pt_ >= 0:
            pbi, pkc = divmod(pt_, KC)
            pg2, _ = blocks[pbi]
            if pkc == 0:
                cpa = psum_ctx.tile([DH + 1, 512], F32, tag="ctx", name="cpa")
                cpb = psum_ctx.tile([DH + 1, 512], F32, tag="ctx", name="cpb")
                cur_cp[pbi] = (cpa, cpb)
            cp0, cp1 = cur_cp[pbi]
            if pbi == 0:
                need(1, f"v{pkc}")
            ex = pend_ex.pop(pt_)
            nc.tensor.matmul(cp0, v_sb[pkc][:, 2 * pg2, :], ex[:, 0, :],
                             start=(pkc == 0), stop=(pkc == KC - 1))
            nc.tensor.matmul(cp1, v_sb[pkc][:, 2 * pg2 + 1, :], ex[:, 1, :],
                             start=(pkc == 0), stop=(pkc == KC - 1))
            if pkc == KC - 1:
                epilogue_a(pbi, cp0, cp1)
                del cur_cp[pbi]
        if (t % KC) in (9, 13) and post:
            post.pop(0)()
        drive(2)

    while post:
        post.pop(0)()
    for idx in range(2):
        while gens[idx] is not None:
            try:
                next(gens[idx])
            except StopIteration:
                gens[idx] = None

    if dbg_aps:
        nc.sync.dma_start(out=dbg_aps["xt"], in_=xt)
        nc.sync.dma_start(out=dbg_aps["qt0"], in_=qts[0])
        nc.sync.dma_start(out=dbg_aps["kt0"], in_=kts[0])
        nc.sync.dma_start(out=dbg_aps["v0"], in_=v_sb[0])
        nc.sync.dma_start(out=dbg_aps["ex00"], in_=dbg_ex)
        nc.sync.dma_start(out=dbg_aps["ctxs0"], in_=dbg_ctxs)
        nc.sync.dma_start(out=dbg_aps["rec0"], in_=dbg_rec)
        nc.sync.dma_start(out=dbg_aps["bc0"], in_=dbg_bc)


def build_program():
    nc = bacc.Bacc("TRN2", target_bir_lowering=False, debug=False)
    x = nc.dram_tensor("x", [S, H], F32, kind="ExternalInput").ap()
    mask = nc.dram_tensor("mask", [S], F32, kind="ExternalInput").ap()
    wq = nc.dram_tensor("wq", [DPC, H], F32, kind="ExternalInput").ap()
    bq = nc.dram_tensor("bq", [DPC], F32, kind="ExternalInput").ap()
    wk = nc.dram_tensor("wk", [DPC, H], F32, kind="ExternalInput").ap()
    bk = nc.dram_tensor("bk", [DPC], F32, kind="ExternalInput").ap()
    wv = nc.dram_tensor("wv", [DPC, H], F32, kind="ExternalInput").ap()
    bv = nc.dram_tensor("bv", [DPC], F32, kind="ExternalInput").ap()
    outT = nc.dram_tensor("outT", [DPC, S], F32, kind="ExternalOutput").ap()

    import os
    if os.environ.get("K_DEBUG"):
        nc._dbg_aps = {
            "xt": nc.dram_tensor("xt_dbg", [128, FC, S], BF16,
                                 kind="ExternalOutput").ap(),
            "qt0": nc.dram_tensor("qt0_dbg", [128, S], BF16,
                                  kind="ExternalOutput").ap(),
            "kt0": nc.dram_tensor("kt0_dbg", [128, S], BF16,
                                  kind="ExternalOutput").ap(),
            "v0": nc.dram_tensor("v0_dbg", [128, HPC, DH + 1], BF16,
                                 kind="ExternalOutput").ap(),
            "ex00": nc.dram_tensor("ex00_dbg", [128, 2, 512], BF16,
                                   kind="ExternalOutput").ap(),
            "ctxs0": nc.dram_tensor("ctxs0_dbg", [DH + 1, 512], F32,
                                    kind="ExternalOutput").ap(),
            "rec0": nc.dram_tensor("rec0_dbg", [1, 512], F32,
                                   kind="ExternalOutput").ap(),
            "bc0": nc.dram_tensor("bc0_dbg", [DH + 1, 512], F32,
                                  kind="ExternalOutput").ap(),
        }

    from contextlib import ExitStack
    with tile.TileContext(nc) as tc:
        with ExitStack() as ctx:
            _emit(ctx, tc, nc, x, mask, wq, bq, wk, bk, wv, bv, outT)
    nc.compile()
    return nc


_NC_CACHE = None


def make_in_maps(hidden_states, attention_mask, Wq, bq, Wk, bk, Wv, bv):
    hs = np.asarray(hidden_states, dtype=np.float32)
    am = np.asarray(attention_mask, dtype=np.float32)
    ws = {k: np.asarray(v, dtype=np.float32)
          for k, v in (("wq", Wq), ("bq", bq), ("wk", Wk),
                       ("bk", bk), ("wv", Wv), ("bv", bv))}
    in_maps = []
    for c in range(N_CORES):
        b, g = divmod(c, 2)
        sl = slice(g * DPC, (g + 1) * DPC)
        in_maps.append({
            "x": np.ascontiguousarray(hs[:, b, :]),
            "mask": np.ascontiguousarray(am[b, 0, 0, :]),
            "wq": np.ascontiguousarray(ws["wq"][sl]),
            "bq": np.ascontiguousarray(ws["bq"][sl]),
            "wk": np.ascontiguousarray(ws["wk"][sl]),
            "bk": np.ascontiguousarray(ws["bk"][sl]),
            "wv": np.ascontiguousarray(ws["wv"][sl]),
            "bv": np.ascontiguousarray(ws["bv"][sl]),
        })
    return in_maps


def gather_out(results):
    out = np.empty((S, B, H), np.float32)
    for c in range(N_CORES):
        b, g = divmod(c, 2)
        out[:, b, g * DPC:(g + 1) * DPC] = results[c]["outT"].T
    return out


def kernel(hidden_states, attention_mask, Wq, bq, Wk, bk, Wv, bv):
    global _NC_CACHE
    if _NC_CACHE is None:
        _NC_CACHE = build_program()
    in_maps = make_in_maps(hidden_states, attention_mask,
                           Wq, bq, Wk, bk, Wv, bv)
    res = run_bass_kernel_spmd(_NC_CACHE, in_maps, list(range(N_CORES)))
    return gather_out(res.results)
